# revision 1
# baseline (speedup 1.0000x reference)
"""Trainium2 Bass kernel for nn_DecoderLayer (GNN message passing decoder layer).

Math (per reference):
  seq_j = seq_emb[edge_idx] * ar_mask[..., None]
  x = concat([h_i, h_j, edge_h, seq_j], -1)            # [res,k,4h]
  msg = gelu(x @ mW1 + mb1); msg = gelu(msg @ mW2 + mb2); msg = msg @ mW3 + mb3
  agg = msg.sum(1)
  h = LN(node_h + agg) * g1 + b1
  ff = gelu(h @ fW1 + fb1) @ fW2 + fb2
  h = LN(h + ff) * g2 + b2

Strategy (8-way data parallel over the residue dim, no collectives):
  - mm1 is decomposed: x@mW1 = h_i@Wa + h_j@Wb + edge_h@Wc + seq_j@Wd.
    h_j@Wb and seq_emb@Wd are precomputed per *global* node into a fused
    bf16 gather table [8192, 256]; per-edge rows are fetched with
    dma_gather and transpose-accumulated into PSUM via identity matmuls.
  - k-reduction is moved before mm3 (linearity): 48x less mm3 work.
  - activations live feature-major ("T layout", [feat, rows]) so every
    matmul uses weights as lhsT directly with zero transposes.
  - edge_h is passed host-pre-transposed [128, E] and fed to the PE as
    float32r (full-rate fp32 moving operand).
"""

import os
import sys

sys.path.insert(0, "/opt/trn_rl_repo")

import numpy as np
import ml_dtypes

import concourse.bacc as bacc
import concourse.bass as bass
import concourse.mybir as mybir
import concourse.tile as tile
from concourse import bass_utils

BF16 = ml_dtypes.bfloat16
F32 = mybir.dt.float32
F32R = mybir.dt.float32r
BF = mybir.dt.bfloat16
I16 = mybir.dt.int16

RES, KK, H = 8192, 48, 128
N_CORES = 8
RT = 384  # rows per psum tile (8 nodes x 48 edges)


def build_nc(n_glob, n_loc, num_devices, chunk_tiles=8):
    """Build the bass program for one core holding n_loc nodes of an
    n_glob-node graph. All sizes in nodes; n_loc % 128 == 0, n_glob % 128 == 0."""
    stage = os.environ.get("KM_STAGE", "full")
    E = n_loc * KK
    assert E % RT == 0
    T = E // RT  # number of 384-row tiles
    n_ch = (T + chunk_tiles - 1) // chunk_tiles
    assert T % n_ch == 0
    cht = T // n_ch  # tiles per chunk
    ch_rows = cht * RT  # rows per chunk (gather num_idxs)
    nblk = n_loc // 128  # node blocks
    gblk = n_glob // 128  # global node blocks (table build)

    nc = bacc.Bacc("TRN2", target_bir_lowering=False, debug=False,
                   num_devices=num_devices)

    def din(name, shape, dt):
        return nc.dram_tensor(name, shape, dt, kind="ExternalInput")

    edge_hT = din("edge_hT", [H, E], F32)
    idx16 = din("idx16", [128, E // 16], I16)
    maskc = din("maskc", [128, 3 * T], F32)
    node_hT = din("node_hT", [H, n_glob], F32)
    seqT = din("seqT", [H, n_glob], F32)
    nhl = din("nhl", [128, nblk, H], F32)
    wa = din("wa", [H, H], mybir.dt.bfloat16)
    wb = din("wb", [H, H], mybir.dt.bfloat16)
    wc = din("wc", [H, H], mybir.dt.bfloat16)
    wd = din("wd", [H, H], mybir.dt.bfloat16)
    w2 = din("w2", [H, H], mybir.dt.bfloat16)
    w3 = din("w3", [H, H], mybir.dt.bfloat16)
    fw1 = din("fw1", [H, 4 * H], mybir.dt.bfloat16)
    fw2 = din("fw2", [H, 4, H], mybir.dt.bfloat16)
    ident = din("ident", [128, 128], mybir.dt.bfloat16)
    mb1c = din("mb1c", [H, 1], F32)
    mb2c = din("mb2c", [H, 1], F32)
    mb3x48 = din("mb3x48", [H, 1], F32)
    fb1c = din("fb1c", [H, 4], F32)
    fb2c = din("fb2c", [H, 1], F32)
    g1bc = din("g1bc", [128, H], mybir.dt.bfloat16)
    b1bc = din("b1bc", [128, H], mybir.dt.bfloat16)
    g2bc = din("g2bc", [128, H], F32)
    b2bc = din("b2bc", [128, H], F32)
    out = nc.dram_tensor("out", [n_loc, H], F32, kind="ExternalOutput")

    GELU = mybir.ActivationFunctionType.Gelu
    IDENT = mybir.ActivationFunctionType.Identity
    COPY = mybir.ActivationFunctionType.Copy
    SQRT = mybir.ActivationFunctionType.Sqrt
    AX = mybir.AxisListType.X
    SUB = mybir.AluOpType.subtract
    MUL = mybir.AluOpType.mult

    with tile.TileContext(nc) as tc:
        with tc.tile_pool(name="singles", bufs=1) as sg, \
             tc.tile_pool(name="dram", bufs=1, space="DRAM") as dp:
            # ---- resident tiles ----
            s_idx = sg.tile([128, E // 16], I16)
            nc.sync.dma_start(out=s_idx[:], in_=idx16.ap())
            s_maskc = sg.tile([128, 3 * T], F32)
            nc.sync.dma_start(out=s_maskc[:], in_=maskc.ap())
            s_nhl = sg.tile([128, nblk, H], F32)
            nc.sync.dma_start(out=s_nhl[:], in_=nhl.ap())
            s_wa = sg.tile([H, H], mybir.dt.bfloat16)
            nc.sync.dma_start(out=s_wa[:], in_=wa.ap())
            s_wb = sg.tile([H, H], mybir.dt.bfloat16)
            nc.sync.dma_start(out=s_wb[:], in_=wb.ap())
            s_wc = sg.tile([H, H], mybir.dt.bfloat16)
            nc.sync.dma_start(out=s_wc[:], in_=wc.ap())
            s_wd = sg.tile([H, H], mybir.dt.bfloat16)
            nc.sync.dma_start(out=s_wd[:], in_=wd.ap())
            s_w2 = sg.tile([H, H], mybir.dt.bfloat16)
            nc.sync.dma_start(out=s_w2[:], in_=w2.ap())
            s_w3 = sg.tile([H, H], mybir.dt.bfloat16)
            nc.sync.dma_start(out=s_w3[:], in_=w3.ap())
            s_fw1 = sg.tile([H, 4 * H], mybir.dt.bfloat16)
            nc.sync.dma_start(out=s_fw1[:], in_=fw1.ap())
            s_fw2 = sg.tile([H, 4, H], mybir.dt.bfloat16)
            nc.sync.dma_start(out=s_fw2[:], in_=fw2.ap())
            s_id = sg.tile([128, 128], mybir.dt.bfloat16)
            nc.sync.dma_start(out=s_id[:], in_=ident.ap())
            s_mb1c = sg.tile([H, 1], F32)
            nc.sync.dma_start(out=s_mb1c[:], in_=mb1c.ap())
            s_mb2c = sg.tile([H, 1], F32)
            nc.sync.dma_start(out=s_mb2c[:], in_=mb2c.ap())
            s_mb3x48 = sg.tile([H, 1], F32)
            nc.sync.dma_start(out=s_mb3x48[:], in_=mb3x48.ap())
            s_fb1c = sg.tile([H, 4], F32)
            nc.sync.dma_start(out=s_fb1c[:], in_=fb1c.ap())
            s_fb2c = sg.tile([H, 1], F32)
            nc.sync.dma_start(out=s_fb2c[:], in_=fb2c.ap())
            s_g1bc = sg.tile([128, H], mybir.dt.bfloat16)
            nc.sync.dma_start(out=s_g1bc[:], in_=g1bc.ap())
            s_b1bc = sg.tile([128, H], mybir.dt.bfloat16)
            nc.sync.dma_start(out=s_b1bc[:], in_=b1bc.ap())
            s_g2bc = sg.tile([128, H], F32)
            nc.sync.dma_start(out=s_g2bc[:], in_=g2bc.ap())
            s_b2bc = sg.tile([128, H], F32)
            nc.sync.dma_start(out=s_b2bc[:], in_=b2bc.ap())
            s_eps = sg.tile([128, 1], F32)
            nc.vector.memset(s_eps[:], 1e-5)

            s_nhTl = sg.tile([128, n_loc], mybir.dt.bfloat16)
            s_aggT = sg.tile([128, n_loc], F32)
            s_aggTb = sg.tile([128, n_loc], mybir.dt.bfloat16)
            s_a2Tb = sg.tile([128, n_loc], mybir.dt.bfloat16)
            s_h1T = sg.tile([128, n_loc], mybir.dt.bfloat16)
            s_h1rm = sg.tile([128, nblk, H], mybir.dt.bfloat16)

            table = dp.tile([n_glob, 256], mybir.dt.bfloat16)

            # ---- phase 1: gather table + nodeA precompute ----
            with tc.tile_pool(name="p1s", bufs=2) as p1s, \
                 tc.tile_pool(name="p1p", bufs=2, space="PSUM") as p1p:
                nhT_bf = p1s.tile([128, n_glob], mybir.dt.bfloat16, tag="big1")
                nc.gpsimd.dma_start(out=nhT_bf[:], in_=node_hT.ap())  # cast
                seT_bf = p1s.tile([128, n_glob], mybir.dt.bfloat16, tag="big2")
                nc.gpsimd.dma_start(out=seT_bf[:], in_=seqT.ap())  # cast

                nc.vector.tensor_copy(out=s_nhTl[:], in_=nhT_bf[:, 0:n_loc])
                for b in range(gblk):
                    ps = p1p.tile([128, 256], F32, tag="tps")
                    nc.tensor.matmul(out=ps[:, 0:128],
                                     lhsT=nhT_bf[:, 128 * b:128 * (b + 1)],
                                     rhs=s_wb[:], start=True, stop=True)
                    nc.tensor.matmul(out=ps[:, 128:256],
                                     lhsT=seT_bf[:, 128 * b:128 * (b + 1)],
                                     rhs=s_wd[:], start=True, stop=True)
                    tb = p1s.tile([128, 256], mybir.dt.bfloat16, tag="tb")
                    nc.scalar.activation(out=tb[:], in_=ps[:], func=COPY)
                    nc.sync.dma_start(out=table[128 * b:128 * (b + 1), :],
                                      in_=tb[:])

            if stage == "p1":
                for b in range(nblk):
                    nc.sync.dma_start(out=out.ap()[128 * b:128 * (b + 1), :],
                                      in_=s_nhl[:, b, :])
            # ---- phase 2: main edge loop ----
            if stage in ("p2", "p3a", "full"):
              with tc.tile_pool(name="p2g", bufs=2) as p2g, \
                   tc.tile_pool(name="p2e", bufs=2) as p2e, \
                   tc.tile_pool(name="p2s", bufs=3) as p2s, \
                   tc.tile_pool(name="pp1", bufs=2, space="PSUM") as pp1, \
                   tc.tile_pool(name="pp2", bufs=2, space="PSUM") as pp2:
                  for ch in range(n_ch):
                      g = p2g.tile([128, 3 * cht, 256], mybir.dt.bfloat16,
                                   tag="g")
                      nc.gpsimd.dma_gather(
                          out_ap=g[:],
                          in_ap=table[:],
                          idxs_ap=s_idx[:, (ch_rows // 16) * ch:
                                        (ch_rows // 16) * (ch + 1)],
                          num_idxs=ch_rows,
                          num_idxs_reg=ch_rows,
                          elem_size=256,
                          single_packet=False,
                      )
                      e = p2e.tile([128, ch_rows], mybir.dt.bfloat16, tag="e")
                      nc.gpsimd.dma_start(out=e[:],
                                          in_=edge_hT.ap()[:, ch_rows * ch:
                                                           ch_rows * (ch + 1)])
                      for tt in range(cht):
                          t = ch * cht + tt
                          for c in range(3):
                              sub = 3 * tt + c
                              nc.vector.tensor_scalar_mul(
                                  out=g[:, sub, 128:256],
                                  in0=g[:, sub, 128:256],
                                  scalar1=s_maskc[:, 3 * t + c:3 * t + c + 1])
                          ps1 = pp1.tile([128, RT], F32, tag="ps1")
                          nc.tensor.matmul(
                              out=ps1[:],
                              lhsT=s_wc[:],
                              rhs=e[:, RT * tt:RT * (tt + 1)],
                              start=True, stop=False)
                          nb = s_nhTl[:, 8 * t:8 * t + 8]
                          rep = bass.AP(tensor=nb.tensor, offset=nb.offset,
                                        ap=[nb.ap[0], nb.ap[1], [0, KK]])
                          nc.tensor.matmul(out=ps1[:], lhsT=s_wa[:], rhs=rep,
                                           start=False, stop=False)
                          for c in range(3):
                              sub = 3 * tt + c
                              nc.tensor.matmul(out=ps1[:, 128 * c:128 * (c + 1)],
                                               lhsT=g[:, sub, 0:128],
                                               rhs=s_id[:],
                                               start=False, stop=False)
                              nc.tensor.matmul(out=ps1[:, 128 * c:128 * (c + 1)],
                                               lhsT=g[:, sub, 128:256],
                                               rhs=s_id[:],
                                               start=False, stop=(c == 2))
                          t2 = p2s.tile([128, RT], mybir.dt.bfloat16, tag="t2")
                          nc.scalar.activation(out=t2[:], in_=ps1[:], func=GELU,
                                               bias=s_mb1c[:])
                          ps3 = pp2.tile([128, RT], F32, tag="ps3")
                          nc.tensor.matmul(out=ps3[:], lhsT=s_w2[:], rhs=t2[:],
                                           start=True, stop=True)
                          t4 = p2s.tile([128, RT], mybir.dt.bfloat16, tag="t4")
                          nc.scalar.activation(out=t4[:], in_=ps3[:], func=GELU,
                                               bias=s_mb2c[:])
                          nc.vector.reduce_sum(
                              out=s_aggT[:, 8 * t:8 * (t + 1)],
                              in_=t4[:].rearrange("p (n k) -> p n k", k=KK),
                              axis=AX)

            if stage == "p2":
                for b in range(nblk):
                    nc.sync.dma_start(out=out.ap()[128 * b:128 * (b + 1), :],
                                      in_=s_aggT[:, 128 * b:128 * (b + 1)])
            # ---- phase 3: mm3, LN1, FF, LN2, output ----
            if stage in ("p3a", "full"):
              with tc.tile_pool(name="p3s", bufs=3) as p3s, \
                   tc.tile_pool(name="p3o", bufs=3) as p3o, \
                   tc.tile_pool(name="pp3", bufs=4, space="PSUM") as pp3, \
                   tc.tile_pool(name="pp4", bufs=2, space="PSUM") as pp4:
                  nh_half = max(1, n_loc // 512)  # halves of <=512 nodes
                  hw_ = min(512, n_loc)  # nodes per half
                  # cast aggT -> bf16
                  for hh in range(nh_half):
                      nc.scalar.activation(out=s_aggTb[:, hw_ * hh:hw_ * (hh + 1)],
                                           in_=s_aggT[:, hw_ * hh:hw_ * (hh + 1)],
                                           func=COPY)
                  # mm3 + mb3*48 -> agg2T bf16
                  for hh in range(nh_half):
                      psm = pp3.tile([128, hw_], F32, tag="p3t")
                      nc.tensor.matmul(out=psm[:], lhsT=s_w3[:],
                                       rhs=s_aggTb[:, hw_ * hh:hw_ * (hh + 1)],
                                       start=True, stop=True)
                      nc.scalar.activation(out=s_a2Tb[:, hw_ * hh:hw_ * (hh + 1)],
                                           in_=psm[:], func=IDENT,
                                           bias=s_mb3x48[:])
                  # LN1 per 128-node block
                  for b in range(nblk):
                      psrm = pp3.tile([128, 128], F32, tag="p3t")
                      nc.tensor.matmul(out=psrm[:],
                                       lhsT=s_a2Tb[:, 128 * b:128 * (b + 1)],
                                       rhs=s_id[:], start=True, stop=True)
                      x1 = p3s.tile([128, 128], F32, tag="x1")
                      nc.vector.tensor_add(out=x1[:], in0=psrm[:],
                                           in1=s_nhl[:, b, :])
                      st = p3s.tile([128, 6], F32, tag="st")
                      nc.vector.bn_stats(out=st[:], in_=x1[:])
                      mv = p3s.tile([128, 2], F32, tag="mv")
                      nc.vector.bn_aggr(out=mv[:], in_=st[:])
                      sd = p3s.tile([128, 1], F32, tag="sd")
                      nc.scalar.activation(out=sd[:], in_=mv[:, 1:2], func=SQRT,
                                           bias=s_eps[:])
                      rstd = p3s.tile([128, 1], F32, tag="rstd")
                      nc.vector.reciprocal(out=rstd[:], in_=sd[:])
                      xn = p3s.tile([128, 128], mybir.dt.bfloat16, tag="xn")
                      nc.vector.tensor_scalar(out=xn[:], in0=x1[:],
                                              scalar1=mv[:, 0:1], scalar2=rstd[:],
                                              op0=SUB, op1=MUL)
                      tb1 = p3s.tile([128, 128], mybir.dt.bfloat16, tag="tb1")
                      nc.vector.tensor_mul(out=tb1[:], in0=xn[:], in1=s_g1bc[:])
                      nc.vector.tensor_add(out=s_h1rm[:, b, :], in0=tb1[:],
                                           in1=s_b1bc[:])
                      psT = pp3.tile([128, 128], F32, tag="p3t")
                      nc.tensor.matmul(out=psT[:], lhsT=s_h1rm[:, b, :],
                                       rhs=s_id[:], start=True, stop=True)
                      nc.scalar.activation(out=s_h1T[:, 128 * b:128 * (b + 1)],
                                           in_=psT[:], func=COPY)
                  if stage == "p3a":
                      for b in range(nblk):
                          nc.gpsimd.dma_start(
                              out=out.ap()[128 * b:128 * (b + 1), :],
                              in_=s_h1rm[:, b, :])
                  # FF + LN2 per half
                  for hh in range(nh_half if stage == "full" else 0):
                      us = []
                      for fc in range(4):
                          psf = pp3.tile([128, hw_], F32, tag="p3t")
                          nc.tensor.matmul(out=psf[:],
                                           lhsT=s_fw1[:, 128 * fc:128 * (fc + 1)],
                                           rhs=s_h1T[:, hw_ * hh:hw_ * (hh + 1)],
                                           start=True, stop=True)
                          u = p3s.tile([128, hw_], mybir.dt.bfloat16,
                                       tag=f"u{fc}")
                          nc.scalar.activation(out=u[:], in_=psf[:], func=GELU,
                                               bias=s_fb1c[:, fc:fc + 1])
                          us.append(u)
                      psf2 = pp4.tile([128, hw_], F32, tag="psf2")
                      for fc in range(4):
                          nc.tensor.matmul(out=psf2[:], lhsT=s_fw2[:, fc, :],
                                           rhs=us[fc][:], start=(fc == 0),
                                           stop=(fc == 3))
                      for j in range(hw_ // 128):
                          b = (hw_ // 128) * hh + j
                          ffT = p3s.tile([128, 128], mybir.dt.bfloat16,
                                         tag="ffT")
                          nc.scalar.activation(out=ffT[:],
                                               in_=psf2[:, 128 * j:128 * (j + 1)],
                                               func=IDENT, bias=s_fb2c[:])
                          psr2 = pp3.tile([128, 128], F32, tag="p3t")
                          nc.tensor.matmul(out=psr2[:], lhsT=ffT[:], rhs=s_id[:],
                                           start=True, stop=True)
                          ffrm = p3s.tile([128, 128], mybir.dt.bfloat16,
                                          tag="ffrm")
                          nc.scalar.activation(out=ffrm[:], in_=psr2[:],
                                               func=COPY)
                          x2 = p3s.tile([128, 128], F32, tag="x2")
                          nc.vector.tensor_add(out=x2[:], in0=ffrm[:],
                                               in1=s_h1rm[:, b, :])
                          st2 = p3s.tile([128, 6], F32, tag="st2")
                          nc.vector.bn_stats(out=st2[:], in_=x2[:])
                          mv2 = p3s.tile([128, 2], F32, tag="mv2")
                          nc.vector.bn_aggr(out=mv2[:], in_=st2[:])
                          sd2 = p3s.tile([128, 1], F32, tag="sd2")
                          nc.scalar.activation(out=sd2[:], in_=mv2[:, 1:2],
                                               func=SQRT, bias=s_eps[:])
                          rstd2 = p3s.tile([128, 1], F32, tag="rstd2")
                          nc.vector.reciprocal(out=rstd2[:], in_=sd2[:])
                          xn2 = p3s.tile([128, 128], F32, tag="xn2")
                          nc.vector.tensor_scalar(out=xn2[:], in0=x2[:],
                                                  scalar1=mv2[:, 0:1],
                                                  scalar2=rstd2[:],
                                                  op0=SUB, op1=MUL)
                          tg = p3s.tile([128, 128], F32, tag="tg")
                          nc.vector.tensor_mul(out=tg[:], in0=xn2[:],
                                               in1=s_g2bc[:])
                          ob = p3o.tile([128, 128], F32, tag="ob")
                          nc.vector.tensor_add(out=ob[:], in0=tg[:],
                                               in1=s_b2bc[:])
                          nc.sync.dma_start(out=out.ap()[128 * b:128 * (b + 1), :],
                                            in_=ob[:])

    nc.compile()
    return nc


def prep_core_inputs(inputs, n_glob, n_loc, core):
    """Host-side layout prep for one core. Pure layout/slicing + tiny
    constant broadcasts; no kernel math is done on the host."""
    f32 = np.float32
    n0 = core * n_loc
    E = n_loc * KK
    T = E // RT
    eh = np.ascontiguousarray(
        inputs["edge_h"][n0:n0 + n_loc].reshape(E, H).T).astype(f32)
    # Rotate the global node axis so this core's local nodes come first;
    # gather indices are rotated to match (table row r = global node
    # (n0 + r) % n_glob).
    j = (inputs["edge_idx"][n0:n0 + n_loc].reshape(E) - n0) % n_glob
    idx16 = np.tile(np.ascontiguousarray(j.reshape(E // 16, 16).T), (8, 1)
                    ).astype(np.int16)
    m = inputs["ar_mask"][n0:n0 + n_loc].reshape(E)
    maskc = np.ascontiguousarray(m.reshape(3 * T, 128).T).astype(f32)
    node_hT = np.ascontiguousarray(
        np.roll(inputs["node_h"], -n0, axis=0).T).astype(f32)
    seqT = np.ascontiguousarray(
        np.roll(inputs["seq_emb"], -n0, axis=0).T).astype(f32)
    nhl = np.ascontiguousarray(
        inputs["node_h"][n0:n0 + n_loc].reshape(n_loc // 128, 128, H)
        .transpose(1, 0, 2)).astype(f32)
    mW1 = inputs["mW1"]
    d = {
        "edge_hT": eh, "idx16": idx16, "maskc": maskc,
        "node_hT": node_hT, "seqT": seqT, "nhl": nhl,
        "wa": mW1[0:128].astype(BF16), "wb": mW1[128:256].astype(BF16),
        "wc": mW1[256:384].astype(BF16),
        "wd": mW1[384:512].astype(BF16),
        "w2": inputs["mW2"].astype(BF16), "w3": inputs["mW3"].astype(BF16),
        "fw1": inputs["fW1"].astype(BF16),
        "fw2": np.ascontiguousarray(
            inputs["fW2"].reshape(4, 128, H).transpose(1, 0, 2)).astype(BF16),
        "ident": np.eye(128, dtype=BF16),
        "mb1c": inputs["mb1"].reshape(H, 1).astype(f32),
        "mb2c": inputs["mb2"].reshape(H, 1).astype(f32),
        "mb3x48": (inputs["mb3"] * KK).reshape(H, 1).astype(f32),
        "fb1c": np.ascontiguousarray(
            inputs["fb1"].reshape(4, 128).T).astype(f32),
        "fb2c": inputs["fb2"].reshape(H, 1).astype(f32),
        "g1bc": np.tile(inputs["g1"][None, :], (128, 1)).astype(BF16),
        "b1bc": np.tile(inputs["b1"][None, :], (128, 1)).astype(BF16),
        "g2bc": np.tile(inputs["g2"][None, :], (128, 1)).astype(f32),
        "b2bc": np.tile(inputs["b2"][None, :], (128, 1)).astype(f32),
    }
    return d


_NC_CACHE = {}


def kernel(**inputs):
    inputs = {k: np.asarray(v) for k, v in inputs.items()}
    n_glob = inputs["node_h"].shape[0]
    n_loc = n_glob // N_CORES
    key = (n_glob, n_loc)
    if key not in _NC_CACHE:
        _NC_CACHE[key] = build_nc(n_glob, n_loc, N_CORES)
    nc = _NC_CACHE[key]
    in_maps = [prep_core_inputs(inputs, n_glob, n_loc, c)
               for c in range(N_CORES)]
    res = bass_utils.run_bass_kernel_spmd(nc, in_maps,
                                          core_ids=list(range(N_CORES)))
    return np.concatenate([res.results[c]["out"] for c in range(N_CORES)],
                          axis=0).astype(np.float32)



# revision 3
# speedup vs baseline: 1.5931x; 1.5931x over previous
"""Trainium2 Bass kernel for nn_DecoderLayer (GNN message passing decoder layer).

Math (per reference):
  seq_j = seq_emb[edge_idx] * ar_mask[..., None]
  x = concat([h_i, h_j, edge_h, seq_j], -1)            # [res,k,4h]
  msg = gelu(x @ mW1 + mb1); msg = gelu(msg @ mW2 + mb2); msg = msg @ mW3 + mb3
  agg = msg.sum(1)
  h = LN(node_h + agg) * g1 + b1
  ff = gelu(h @ fW1 + fb1) @ fW2 + fb2
  h = LN(h + ff) * g2 + b2

Strategy (8-way data parallel over the residue dim, no collectives):
  - mm1 decomposed: x@mW1 = h_i@Wa + h_j@Wb + edge_h@Wc + seq_j@Wd.
    h_j@Wb and seq_emb@Wd are precomputed per global node into a fused bf16
    gather table [8192, 256] in DRAM; per-edge rows fetched with dma_gather.
  - dma_gather descriptor generation is the critical resource: it runs on the
    GpSimd Q7 core-pair selected by queue_num.  queue 0 blocks the engine
    sequencer, queues 1-3 are fire-and-forget and run CONCURRENTLY on three
    disjoint core pairs -> gathers round-robin queues 1..3 (needs
    num_swdge_queues=4).
  - Edges are laid out k-major within 3072-edge chunks (64 nodes x 48 k):
    col = k*64 + n.  This aligns every 512-col matmul slice with the
    per-node h_i@Wa broadcast AP and makes the k-reduction a log-tree of
    dense bf16 adds on DVE.
  - ar_mask is folded into the PE transpose of the gathered seq half:
    rhs = diag(mask) instead of identity (diagonal built on DVE from a mask
    column).  No separate mask multiply pass.
  - k-reduction before mm3 (linearity): 48x less mm3 work.
"""

import os
import sys

sys.path.insert(0, "/opt/trn_rl_repo")

import numpy as np
import ml_dtypes

import concourse.bacc as bacc
import concourse.bass as bass
import concourse.mybir as mybir
import concourse.tile as tile
from concourse import bass_utils

BF16 = ml_dtypes.bfloat16
F32 = mybir.dt.float32
BF = mybir.dt.bfloat16
I16 = mybir.dt.int16

RES, KK, H = 8192, 48, 128
N_CORES = 8
CH_NODES = 64                 # nodes per chunk
CH_E = CH_NODES * KK          # 3072 edges per chunk
HC_E = CH_E // 2              # 1536 edges per half-chunk
N_SUB = CH_E // 128           # 24 subtiles of 128 edges per chunk


def build_nc(n_glob, n_loc, num_devices):
    E = n_loc * KK
    n_ch = E // CH_E           # 16 chunks
    nblk = n_loc // 128        # 8 local node blocks
    gblk = n_glob // 128       # 64 global node blocks

    nc = bacc.Bacc("TRN2", target_bir_lowering=False, debug=False,
                   num_devices=num_devices, num_swdge_queues=4)

    def din(name, shape, dt):
        return nc.dram_tensor(name, shape, dt, kind="ExternalInput")

    edge_hT = din("edge_hT", [H, E], BF)            # k-major per chunk
    idx16 = din("idx16", [128, E // 16], I16)       # k-major per chunk
    maskc = din("maskc", [128, E // 128], F32)      # subtile-column-major
    node_hT = din("node_hT", [H, n_glob], BF)       # rotated: local first
    seqT = din("seqT", [H, n_glob], BF)
    nhl = din("nhl", [128, nblk, H], F32)           # local node_h row-major
    wa = din("wa", [H, H], BF)
    wb = din("wb", [H, H], BF)
    wc = din("wc", [H, H], BF)
    wd = din("wd", [H, H], BF)
    w2 = din("w2", [H, H], BF)
    w3 = din("w3", [H, H], BF)
    fw1 = din("fw1", [H, 4 * H], BF)
    fw2 = din("fw2", [H, 4, H], BF)
    ident = din("ident", [128, 128], BF)
    mb1c = din("mb1c", [H, 1], F32)
    mb2c = din("mb2c", [H, 1], F32)
    mb3x48 = din("mb3x48", [H, 1], F32)
    fb1c = din("fb1c", [H, 4], F32)
    fb2c = din("fb2c", [H, 1], F32)
    g1bc = din("g1bc", [128, H], BF)
    b1bc = din("b1bc", [128, H], BF)
    g2bc = din("g2bc", [128, H], F32)
    b2bc = din("b2bc", [128, H], F32)
    out = nc.dram_tensor("out", [n_loc, H], F32, kind="ExternalOutput")

    GELU = mybir.ActivationFunctionType.Gelu
    IDENT = mybir.ActivationFunctionType.Identity
    COPY = mybir.ActivationFunctionType.Copy
    SQRT = mybir.ActivationFunctionType.Sqrt
    SUB = mybir.AluOpType.subtract
    MUL = mybir.AluOpType.mult

    with tile.TileContext(nc) as tc:
        with tc.tile_pool(name="singles", bufs=1) as sg, \
             tc.tile_pool(name="dram", bufs=1, space="DRAM") as dp:
            # ---- small resident tiles ----
            s_idx = sg.tile([128, E // 16], I16)
            nc.sync.dma_start(out=s_idx[:], in_=idx16.ap())
            s_maskc = sg.tile([128, E // 128], F32)
            nc.sync.dma_start(out=s_maskc[:], in_=maskc.ap())
            s_nhl = sg.tile([128, nblk, H], F32)
            nc.sync.dma_start(out=s_nhl[:], in_=nhl.ap())
            s_wa = sg.tile([H, H], BF)
            nc.sync.dma_start(out=s_wa[:], in_=wa.ap())
            s_wb = sg.tile([H, H], BF)
            nc.sync.dma_start(out=s_wb[:], in_=wb.ap())
            s_wc = sg.tile([H, H], BF)
            nc.sync.dma_start(out=s_wc[:], in_=wc.ap())
            s_wd = sg.tile([H, H], BF)
            nc.sync.dma_start(out=s_wd[:], in_=wd.ap())
            s_w2 = sg.tile([H, H], BF)
            nc.sync.dma_start(out=s_w2[:], in_=w2.ap())
            s_w3 = sg.tile([H, H], BF)
            nc.sync.dma_start(out=s_w3[:], in_=w3.ap())
            s_fw1 = sg.tile([H, 4 * H], BF)
            nc.sync.dma_start(out=s_fw1[:], in_=fw1.ap())
            s_fw2 = sg.tile([H, 4, H], BF)
            nc.sync.dma_start(out=s_fw2[:], in_=fw2.ap())
            s_id = sg.tile([128, 128], BF)
            nc.sync.dma_start(out=s_id[:], in_=ident.ap())
            s_mb1c = sg.tile([H, 1], F32)
            nc.sync.dma_start(out=s_mb1c[:], in_=mb1c.ap())
            s_mb2c = sg.tile([H, 1], F32)
            nc.sync.dma_start(out=s_mb2c[:], in_=mb2c.ap())
            s_mb3x48 = sg.tile([H, 1], F32)
            nc.sync.dma_start(out=s_mb3x48[:], in_=mb3x48.ap())
            s_fb1c = sg.tile([H, 4], F32)
            nc.sync.dma_start(out=s_fb1c[:], in_=fb1c.ap())
            s_fb2c = sg.tile([H, 1], F32)
            nc.sync.dma_start(out=s_fb2c[:], in_=fb2c.ap())
            s_g1bc = sg.tile([128, H], BF)
            nc.sync.dma_start(out=s_g1bc[:], in_=g1bc.ap())
            s_b1bc = sg.tile([128, H], BF)
            nc.sync.dma_start(out=s_b1bc[:], in_=b1bc.ap())
            s_g2bc = sg.tile([128, H], F32)
            nc.sync.dma_start(out=s_g2bc[:], in_=g2bc.ap())
            s_b2bc = sg.tile([128, H], F32)
            nc.sync.dma_start(out=s_b2bc[:], in_=b2bc.ap())
            s_eps = sg.tile([128, 1], F32)
            nc.vector.memset(s_eps[:], 1e-5)

            s_aT = sg.tile([128, n_loc], BF)        # (Wa^T h_i) per local node
            s_aggTb = sg.tile([128, n_loc], BF)     # k-sum of msg2, fm bf16
            s_a2Tb = sg.tile([128, n_loc], BF)
            s_h1T = sg.tile([128, n_loc], BF)
            s_h1rm = sg.tile([128, nblk, H], BF)

            table = dp.tile([n_glob, 256], BF)

            # ---- phase 1: gather table + Wa precompute ----
            with tc.tile_pool(name="p1s", bufs=2) as p1s, \
                 tc.tile_pool(name="p1p", bufs=3, space="PSUM") as p1p:
                nhT_bf = p1s.tile([128, n_glob], BF, tag="big1")
                nc.sync.dma_start(out=nhT_bf[:], in_=node_hT.ap())
                seT_bf = p1s.tile([128, n_glob], BF, tag="big2")
                nc.sync.dma_start(out=seT_bf[:], in_=seqT.ap())

                # aT = Wa^T h for local nodes (feature-major)
                for hh in range(n_loc // 512):
                    psa = p1p.tile([128, 512], F32, tag="psa")
                    nc.tensor.matmul(out=psa[:], lhsT=s_wa[:],
                                     rhs=nhT_bf[:, 512 * hh:512 * (hh + 1)],
                                     start=True, stop=True)
                    nc.scalar.activation(out=s_aT[:, 512 * hh:512 * (hh + 1)],
                                         in_=psa[:], func=COPY)
                # table rows: node-major [128, 256] per block, u | v halves
                for b in range(gblk):
                    ps = p1p.tile([128, 256], F32, tag="tps")
                    nc.tensor.matmul(out=ps[:, 0:128],
                                     lhsT=nhT_bf[:, 128 * b:128 * (b + 1)],
                                     rhs=s_wb[:], start=True, stop=True)
                    nc.tensor.matmul(out=ps[:, 128:256],
                                     lhsT=seT_bf[:, 128 * b:128 * (b + 1)],
                                     rhs=s_wd[:], start=True, stop=True)
                    tb = p1s.tile([128, 256], BF, tag="tb")
                    if b % 2 == 0:
                        nc.scalar.activation(out=tb[:], in_=ps[:], func=COPY)
                    else:
                        nc.vector.tensor_copy(out=tb[:], in_=ps[:])
                    nc.scalar.dma_start(out=table[128 * b:128 * (b + 1), :],
                                        in_=tb[:])

            # ---- phase 2: main edge loop, k-major chunks ----
            with tc.tile_pool(name="p2g", bufs=5) as p2g, \
                 tc.tile_pool(name="p2e", bufs=3) as p2e, \
                 tc.tile_pool(name="p2d", bufs=2) as p2d, \
                 tc.tile_pool(name="p2t", bufs=3) as p2t, \
                 tc.tile_pool(name="p2r", bufs=2) as p2r, \
                 tc.tile_pool(name="pp1", bufs=2, space="PSUM") as pp1, \
                 tc.tile_pool(name="ppw", bufs=2, space="PSUM") as ppw:
                for ch in range(n_ch):
                    g = p2g.tile([128, N_SUB, 256], BF, tag="g")
                    nc.gpsimd.dma_gather(
                        out_ap=g[:],
                        in_ap=table[:],
                        idxs_ap=s_idx[:, (CH_E // 16) * ch:
                                      (CH_E // 16) * (ch + 1)],
                        num_idxs=CH_E,
                        num_idxs_reg=CH_E,
                        elem_size=256,
                        single_packet=False,
                        queue_num=1 + ch % 3,
                    )
                    e = p2e.tile([128, CH_E], BF, tag="e")
                    nc.sync.dma_start(out=e[:],
                                      in_=edge_hT.ap()[:, CH_E * ch:
                                                       CH_E * (ch + 1)])
                    # diag(mask) tiles for this chunk's 24 subtiles
                    dg = p2d.tile([128, N_SUB, 128], BF, tag="dg")
                    for s in range(N_SUB):
                        nc.vector.tensor_scalar_mul(
                            out=dg[:, s, :], in0=s_id[:],
                            scalar1=s_maskc[:, N_SUB * ch + s:
                                            N_SUB * ch + s + 1])

                    t4 = p2t.tile([128, CH_E], BF, tag="t4")
                    for hc in range(2):
                        ps1 = pp1.tile([128, 3, 512], F32, tag="ps1")
                        e0 = HC_E * hc  # edge col offset within chunk
                        for b in range(3):
                            nc.tensor.matmul(
                                out=ps1[:, b, :], lhsT=s_wc[:],
                                rhs=e[:, e0 + 512 * b:e0 + 512 * (b + 1)],
                                start=True, stop=False)
                        # h_i @ Wa broadcast: [[0,8],[1,64]] per 512 cols
                        na = s_aT[:, CH_NODES * ch:CH_NODES * (ch + 1)]
                        rep = bass.AP(tensor=na.tensor, offset=na.offset,
                                      ap=[na.ap[0], [0, 8], na.ap[1]])
                        for b in range(3):
                            nc.tensor.matmul(out=ps1[:, b, :], lhsT=s_id[:],
                                             rhs=rep, start=False, stop=False)
                        for sub in range(12):
                            gsub = 12 * hc + sub
                            bank = sub // 4
                            col = 128 * (sub % 4)
                            nc.tensor.matmul(
                                out=ps1[:, bank, col:col + 128],
                                lhsT=g[:, gsub, 0:128], rhs=s_id[:],
                                start=False, stop=False)
                            nc.tensor.matmul(
                                out=ps1[:, bank, col:col + 128],
                                lhsT=g[:, gsub, 128:256], rhs=dg[:, gsub, :],
                                start=False, stop=True)
                        t2 = p2t.tile([128, HC_E], BF, tag="t2")
                        nc.scalar.activation(out=t2[:], in_=ps1[:], func=GELU,
                                             bias=s_mb1c[:])
                        for b in range(3):
                            psw = ppw.tile([128, 512], F32, tag="psw")
                            nc.tensor.matmul(out=psw[:], lhsT=s_w2[:],
                                             rhs=t2[:, 512 * b:512 * (b + 1)],
                                             start=True, stop=True)
                            nc.scalar.activation(
                                out=t4[:, e0 + 512 * b:e0 + 512 * (b + 1)],
                                in_=psw[:], func=GELU, bias=s_mb2c[:])
                    # k-major tree reduce: 48 -> 24 -> 12 -> 6 -> 3 -> 1
                    r1 = p2r.tile([128, HC_E], BF, tag="r1")
                    nc.vector.tensor_add(out=r1[:], in0=t4[:, 0:HC_E],
                                         in1=t4[:, HC_E:CH_E])
                    nc.vector.tensor_add(out=r1[:, 0:768], in0=r1[:, 0:768],
                                         in1=r1[:, 768:1536])
                    nc.vector.tensor_add(out=r1[:, 0:384], in0=r1[:, 0:384],
                                         in1=r1[:, 384:768])
                    nc.vector.tensor_add(out=r1[:, 0:192], in0=r1[:, 0:192],
                                         in1=r1[:, 192:384])
                    nc.vector.tensor_add(out=r1[:, 0:64], in0=r1[:, 0:64],
                                         in1=r1[:, 64:128])
                    nc.vector.tensor_add(out=s_aggTb[:, CH_NODES * ch:
                                                     CH_NODES * (ch + 1)],
                                         in0=r1[:, 0:64], in1=r1[:, 128:192])

            # ---- phase 3: mm3, LN1, FF, LN2, output ----
            with tc.tile_pool(name="p3s", bufs=3) as p3s, \
                 tc.tile_pool(name="p3o", bufs=3) as p3o, \
                 tc.tile_pool(name="pp3", bufs=4, space="PSUM") as pp3, \
                 tc.tile_pool(name="pp4", bufs=2, space="PSUM") as pp4:
                nh_half = n_loc // 512
                # mm3 + 48*mb3 -> a2Tb bf16
                for hh in range(nh_half):
                    psm = pp3.tile([128, 512], F32, tag="p3t")
                    nc.tensor.matmul(out=psm[:], lhsT=s_w3[:],
                                     rhs=s_aggTb[:, 512 * hh:512 * (hh + 1)],
                                     start=True, stop=True)
                    nc.scalar.activation(out=s_a2Tb[:, 512 * hh:512 * (hh + 1)],
                                         in_=psm[:], func=IDENT,
                                         bias=s_mb3x48[:])
                # LN1 per 128-node block
                for b in range(nblk):
                    psrm = pp3.tile([128, 128], F32, tag="p3t")
                    nc.tensor.matmul(out=psrm[:],
                                     lhsT=s_a2Tb[:, 128 * b:128 * (b + 1)],
                                     rhs=s_id[:], start=True, stop=True)
                    x1 = p3s.tile([128, 128], F32, tag="x1")
                    nc.vector.tensor_add(out=x1[:], in0=psrm[:],
                                         in1=s_nhl[:, b, :])
                    st = p3s.tile([128, 6], F32, tag="st")
                    nc.vector.bn_stats(out=st[:], in_=x1[:])
                    mv = p3s.tile([128, 2], F32, tag="mv")
                    nc.vector.bn_aggr(out=mv[:], in_=st[:])
                    sd = p3s.tile([128, 1], F32, tag="sd")
                    nc.scalar.activation(out=sd[:], in_=mv[:, 1:2], func=SQRT,
                                         bias=s_eps[:])
                    rstd = p3s.tile([128, 1], F32, tag="rstd")
                    nc.vector.reciprocal(out=rstd[:], in_=sd[:])
                    xn = p3s.tile([128, 128], BF, tag="xn")
                    nc.vector.tensor_scalar(out=xn[:], in0=x1[:],
                                            scalar1=mv[:, 0:1], scalar2=rstd[:],
                                            op0=SUB, op1=MUL)
                    tb1 = p3s.tile([128, 128], BF, tag="tb1")
                    nc.vector.tensor_mul(out=tb1[:], in0=xn[:], in1=s_g1bc[:])
                    nc.vector.tensor_add(out=s_h1rm[:, b, :], in0=tb1[:],
                                         in1=s_b1bc[:])
                    psT = pp3.tile([128, 128], F32, tag="p3t")
                    nc.tensor.matmul(out=psT[:], lhsT=s_h1rm[:, b, :],
                                     rhs=s_id[:], start=True, stop=True)
                    nc.scalar.activation(out=s_h1T[:, 128 * b:128 * (b + 1)],
                                         in_=psT[:], func=COPY)
                # FF + LN2 per 512-node half
                for hh in range(nh_half):
                    us = []
                    for fc in range(4):
                        psf = pp3.tile([128, 512], F32, tag="p3t")
                        nc.tensor.matmul(out=psf[:],
                                         lhsT=s_fw1[:, 128 * fc:128 * (fc + 1)],
                                         rhs=s_h1T[:, 512 * hh:512 * (hh + 1)],
                                         start=True, stop=True)
                        u = p3s.tile([128, 512], BF, tag=f"u{fc}")
                        nc.scalar.activation(out=u[:], in_=psf[:], func=GELU,
                                             bias=s_fb1c[:, fc:fc + 1])
                        us.append(u)
                    psf2 = pp4.tile([128, 512], F32, tag="psf2")
                    for fc in range(4):
                        nc.tensor.matmul(out=psf2[:], lhsT=s_fw2[:, fc, :],
                                         rhs=us[fc][:], start=(fc == 0),
                                         stop=(fc == 3))
                    for j in range(4):
                        b = 4 * hh + j
                        ffT = p3s.tile([128, 128], BF, tag="ffT")
                        nc.scalar.activation(out=ffT[:],
                                             in_=psf2[:, 128 * j:128 * (j + 1)],
                                             func=IDENT, bias=s_fb2c[:])
                        psr2 = pp3.tile([128, 128], F32, tag="p3t")
                        nc.tensor.matmul(out=psr2[:], lhsT=ffT[:], rhs=s_id[:],
                                         start=True, stop=True)
                        x2 = p3s.tile([128, 128], F32, tag="x2")
                        nc.vector.tensor_add(out=x2[:], in0=psr2[:],
                                             in1=s_h1rm[:, b, :])
                        st2 = p3s.tile([128, 6], F32, tag="st2")
                        nc.vector.bn_stats(out=st2[:], in_=x2[:])
                        mv2 = p3s.tile([128, 2], F32, tag="mv2")
                        nc.vector.bn_aggr(out=mv2[:], in_=st2[:])
                        sd2 = p3s.tile([128, 1], F32, tag="sd2")
                        nc.scalar.activation(out=sd2[:], in_=mv2[:, 1:2],
                                             func=SQRT, bias=s_eps[:])
                        rstd2 = p3s.tile([128, 1], F32, tag="rstd2")
                        nc.vector.reciprocal(out=rstd2[:], in_=sd2[:])
                        xn2 = p3s.tile([128, 128], F32, tag="xn2")
                        nc.vector.tensor_scalar(out=xn2[:], in0=x2[:],
                                                scalar1=mv2[:, 0:1],
                                                scalar2=rstd2[:],
                                                op0=SUB, op1=MUL)
                        tg = p3s.tile([128, 128], F32, tag="tg")
                        nc.vector.tensor_mul(out=tg[:], in0=xn2[:],
                                             in1=s_g2bc[:])
                        ob = p3o.tile([128, 128], F32, tag="ob")
                        nc.vector.tensor_add(out=ob[:], in0=tg[:],
                                             in1=s_b2bc[:])
                        nc.sync.dma_start(out=out.ap()[128 * b:128 * (b + 1), :],
                                          in_=ob[:])

    nc.compile()
    return nc


def prep_core_inputs(inputs, n_glob, n_loc, core):
    """Host-side layout prep for one core: slicing, k-major reorder within
    chunks, transposes, dtype casts, tiny constant broadcasts. No kernel math
    (no indexing of data tensors by edge_idx) is done on the host."""
    f32 = np.float32
    n0 = core * n_loc
    E = n_loc * KK
    n_ch = E // CH_E

    def kmajor(x):
        # x: [n_loc, KK, ...] -> per 64-node chunk: [KK, 64, ...] -> flat E
        tail = x.shape[2:]
        x = x.reshape(n_ch, CH_NODES, KK, *tail)
        x = x.transpose(0, 2, 1, *range(3, 3 + len(tail)))
        return np.ascontiguousarray(x.reshape(E, *tail))

    eh = kmajor(inputs["edge_h"][n0:n0 + n_loc])          # [E, H] k-major
    eh = np.ascontiguousarray(eh.T).astype(BF16)          # [H, E]
    j = (inputs["edge_idx"][n0:n0 + n_loc].astype(np.int64) - n0) % n_glob
    j = kmajor(j)                                         # [E] k-major
    idx16 = np.tile(np.ascontiguousarray(j.reshape(E // 16, 16).T), (8, 1)
                    ).astype(np.int16)
    m = kmajor(inputs["ar_mask"][n0:n0 + n_loc])          # [E] k-major
    maskc = np.ascontiguousarray(m.reshape(E // 128, 128).T).astype(f32)
    node_hT = np.ascontiguousarray(
        np.roll(inputs["node_h"], -n0, axis=0).T).astype(BF16)
    seqT = np.ascontiguousarray(
        np.roll(inputs["seq_emb"], -n0, axis=0).T).astype(BF16)
    nhl = np.ascontiguousarray(
        inputs["node_h"][n0:n0 + n_loc].reshape(n_loc // 128, 128, H)
        .transpose(1, 0, 2)).astype(f32)
    mW1 = inputs["mW1"]
    d = {
        "edge_hT": eh, "idx16": idx16, "maskc": maskc,
        "node_hT": node_hT, "seqT": seqT, "nhl": nhl,
        "wa": mW1[0:128].astype(BF16), "wb": mW1[128:256].astype(BF16),
        "wc": mW1[256:384].astype(BF16),
        "wd": mW1[384:512].astype(BF16),
        "w2": inputs["mW2"].astype(BF16), "w3": inputs["mW3"].astype(BF16),
        "fw1": inputs["fW1"].astype(BF16),
        "fw2": np.ascontiguousarray(
            inputs["fW2"].reshape(4, 128, H).transpose(1, 0, 2)).astype(BF16),
        "ident": np.eye(128, dtype=BF16),
        "mb1c": inputs["mb1"].reshape(H, 1).astype(f32),
        "mb2c": inputs["mb2"].reshape(H, 1).astype(f32),
        "mb3x48": (inputs["mb3"] * KK).reshape(H, 1).astype(f32),
        "fb1c": np.ascontiguousarray(
            inputs["fb1"].reshape(4, 128).T).astype(f32),
        "fb2c": inputs["fb2"].reshape(H, 1).astype(f32),
        "g1bc": np.tile(inputs["g1"][None, :], (128, 1)).astype(BF16),
        "b1bc": np.tile(inputs["b1"][None, :], (128, 1)).astype(BF16),
        "g2bc": np.tile(inputs["g2"][None, :], (128, 1)).astype(f32),
        "b2bc": np.tile(inputs["b2"][None, :], (128, 1)).astype(f32),
    }
    return d


_NC_CACHE = {}


def kernel(**inputs):
    inputs = {k: np.asarray(v) for k, v in inputs.items()}
    n_glob = inputs["node_h"].shape[0]
    n_loc = n_glob // N_CORES
    key = (n_glob, n_loc)
    if key not in _NC_CACHE:
        _NC_CACHE[key] = build_nc(n_glob, n_loc, N_CORES)
    nc = _NC_CACHE[key]
    in_maps = [prep_core_inputs(inputs, n_glob, n_loc, c)
               for c in range(N_CORES)]
    res = bass_utils.run_bass_kernel_spmd(nc, in_maps,
                                          core_ids=list(range(N_CORES)))
    return np.concatenate([res.results[c]["out"] for c in range(N_CORES)],
                          axis=0).astype(np.float32)


# revision 8
# speedup vs baseline: 2.1567x; 1.3538x over previous
"""Trainium2 Bass kernel for nn_DecoderLayer (GNN message passing decoder layer).

Math (per reference):
  seq_j = seq_emb[edge_idx] * ar_mask[..., None]
  x = concat([h_i, h_j, edge_h, seq_j], -1)            # [res,k,4h]
  msg = gelu(x @ mW1 + mb1); msg = gelu(msg @ mW2 + mb2); msg = msg @ mW3 + mb3
  agg = msg.sum(1)
  h = LN(node_h + agg) * g1 + b1
  ff = gelu(h @ fW1 + fb1) @ fW2 + fb2
  h = LN(h + ff) * g2 + b2

Strategy (8-way data parallel over the residue dim, no collectives):
  - mm1 decomposed: x@mW1 = h_i@Wa + h_j@Wb + edge_h@Wc + seq_j@Wd.
    h_j@Wb and seq_emb@Wd are precomputed per global node into a fused bf16
    gather table [8192, 256] in DRAM; per-edge rows fetched with dma_gather.
  - dma_gather descriptor generation runs on the GpSimd Q7 core-pair selected
    by queue_num; queues 1-3 run concurrently on three disjoint pairs
    (num_swdge_queues=4), tripling gather throughput vs one queue.
  - Edges are k-major within 3072-edge chunks (64 nodes x 48 k): col=k*64+n.
    Aligns 512-col matmul slices with the per-node h_i@Wa broadcast AP and
    makes the k-reduction a log-tree of dense bf16 adds on DVE.
  - ar_mask folded into the PE transpose of the gathered seq half:
    rhs = diag(mask) built on DVE. No separate mask multiply pass.
  - k-reduction before mm3 (linearity): 48x less mm3 work.
"""

import os
import sys

sys.path.insert(0, "/opt/trn_rl_repo")

import numpy as np
import ml_dtypes

import concourse.bacc as bacc
import concourse.bass as bass
import concourse.mybir as mybir
import concourse.tile as tile
from concourse import bass_utils

BF16 = ml_dtypes.bfloat16
F32 = mybir.dt.float32
BF = mybir.dt.bfloat16
I16 = mybir.dt.int16

RES, KK, H = 8192, 48, 128
N_CORES = 8
CH_NODES = 64                 # nodes per chunk
CH_E = CH_NODES * KK          # 3072 edges per chunk
HC_E = CH_E // 2              # 1536 edges per half-chunk
N_SUB = CH_E // 128           # 24 subtiles of 128 edges per chunk


def build_nc(n_glob, n_loc, num_devices):
    E = n_loc * KK
    n_ch = E // CH_E           # 16 chunks
    nblk = n_loc // 128        # 8 local node blocks
    gblk = n_glob // 128       # 64 global node blocks

    nc = bacc.Bacc("TRN2", target_bir_lowering=False, debug=False,
                   num_devices=num_devices, num_swdge_queues=4)

    def din(name, shape, dt):
        return nc.dram_tensor(name, shape, dt, kind="ExternalInput")

    edge_hT = din("edge_hT", [H, E], BF)            # k-major per chunk
    idx16 = din("idx16", [128, E // 16], I16)       # k-major per chunk
    maskc = din("maskc", [128, E // 128], F32)      # subtile-column-major
    node_hT = din("node_hT", [H, n_glob], BF)       # rotated: local first
    seqT = din("seqT", [H, n_glob], BF)
    nhl = din("nhl", [128, nblk, H], F32)           # local node_h row-major
    wa = din("wa", [H, H], BF)
    wb = din("wb", [H, H], BF)
    wc = din("wc", [H, H], BF)
    wd = din("wd", [H, H], BF)
    w2 = din("w2", [H, H], BF)
    w3 = din("w3", [H, H], BF)
    fw1 = din("fw1", [H, 4 * H], BF)
    fw2 = din("fw2", [H, 4, H], BF)
    ident = din("ident", [128, 128], BF)
    mb1c = din("mb1c", [H, 1], F32)
    mb2c = din("mb2c", [H, 1], F32)
    mb3x48 = din("mb3x48", [H, 1], F32)
    fb1c = din("fb1c", [H, 4], F32)
    fb2c = din("fb2c", [H, 1], F32)
    g1bc = din("g1bc", [128, H], BF)
    b1bc = din("b1bc", [128, H], BF)
    g2bc = din("g2bc", [128, H], F32)
    b2bc = din("b2bc", [128, H], F32)
    out = nc.dram_tensor("out", [n_loc, H], F32, kind="ExternalOutput")

    GELU = mybir.ActivationFunctionType.Gelu
    IDENT = mybir.ActivationFunctionType.Identity
    COPY = mybir.ActivationFunctionType.Copy
    SQRT = mybir.ActivationFunctionType.Sqrt
    SUB = mybir.AluOpType.subtract
    MUL = mybir.AluOpType.mult

    with tile.TileContext(nc) as tc:
        with tc.tile_pool(name="singles", bufs=1) as sg, \
             tc.tile_pool(name="dram", bufs=1, space="DRAM") as dp:
            # ---- small resident tiles ----
            s_idx = sg.tile([128, E // 16], I16)
            nc.sync.dma_start(out=s_idx[:], in_=idx16.ap())
            s_maskc = sg.tile([128, E // 128], F32)
            nc.sync.dma_start(out=s_maskc[:], in_=maskc.ap())
            s_nhl = sg.tile([128, nblk, H], F32)
            nc.sync.dma_start(out=s_nhl[:], in_=nhl.ap())
            s_wa = sg.tile([H, H], BF)
            nc.sync.dma_start(out=s_wa[:], in_=wa.ap())
            s_wb = sg.tile([H, H], BF)
            nc.sync.dma_start(out=s_wb[:], in_=wb.ap())
            s_wc = sg.tile([H, H], BF)
            nc.sync.dma_start(out=s_wc[:], in_=wc.ap())
            s_wd = sg.tile([H, H], BF)
            nc.sync.dma_start(out=s_wd[:], in_=wd.ap())
            s_w2 = sg.tile([H, H], BF)
            nc.sync.dma_start(out=s_w2[:], in_=w2.ap())
            s_w3 = sg.tile([H, H], BF)
            nc.sync.dma_start(out=s_w3[:], in_=w3.ap())
            s_fw1 = sg.tile([H, 4 * H], BF)
            nc.sync.dma_start(out=s_fw1[:], in_=fw1.ap())
            s_fw2 = sg.tile([H, 4, H], BF)
            nc.sync.dma_start(out=s_fw2[:], in_=fw2.ap())
            s_id = sg.tile([128, 128], BF)
            nc.sync.dma_start(out=s_id[:], in_=ident.ap())
            s_mb1c = sg.tile([H, 1], F32)
            nc.sync.dma_start(out=s_mb1c[:], in_=mb1c.ap())
            s_mb2c = sg.tile([H, 1], F32)
            nc.sync.dma_start(out=s_mb2c[:], in_=mb2c.ap())
            s_mb3x48 = sg.tile([H, 1], F32)
            nc.sync.dma_start(out=s_mb3x48[:], in_=mb3x48.ap())
            s_fb1c = sg.tile([H, 4], F32)
            nc.sync.dma_start(out=s_fb1c[:], in_=fb1c.ap())
            s_fb2c = sg.tile([H, 1], F32)
            nc.sync.dma_start(out=s_fb2c[:], in_=fb2c.ap())
            s_g1bc = sg.tile([128, H], BF)
            nc.sync.dma_start(out=s_g1bc[:], in_=g1bc.ap())
            s_b1bc = sg.tile([128, H], BF)
            nc.sync.dma_start(out=s_b1bc[:], in_=b1bc.ap())
            s_g2bc = sg.tile([128, H], F32)
            nc.sync.dma_start(out=s_g2bc[:], in_=g2bc.ap())
            s_b2bc = sg.tile([128, H], F32)
            nc.sync.dma_start(out=s_b2bc[:], in_=b2bc.ap())
            s_eps = sg.tile([128, 1], F32)
            nc.vector.memset(s_eps[:], 1e-5)

            s_aT = sg.tile([128, n_loc], BF)        # (Wa^T h_i) per local node
            s_aggTb = sg.tile([128, n_loc], BF)     # k-sum of msg2, fm bf16
            s_a2Tb = sg.tile([128, n_loc], BF)
            s_h1T = sg.tile([128, n_loc], BF)
            s_h1rm = sg.tile([128, nblk, H], BF)

            table = dp.tile([n_glob, 256], BF)

            # ---- phase 1: gather table + Wa precompute ----
            # nhT/seqT loaded in quarters so block-0 compute starts early;
            # table staged in SBUF and written to DRAM in 4 batched DMAs.
            with tc.tile_pool(name="p1s", bufs=1) as p1s, \
                 tc.tile_pool(name="p1c", bufs=4) as p1c, \
                 tc.tile_pool(name="p1p", bufs=3, space="PSUM") as p1p:
                nhT_bf = p1s.tile([128, n_glob], BF, tag="big1")
                seT_bf = p1s.tile([128, n_glob], BF, tag="big2")
                qn = n_glob // 4
                for qq in range(4):
                    nc.sync.dma_start(out=nhT_bf[:, qn * qq:qn * (qq + 1)],
                                      in_=node_hT.ap()[:, qn * qq:qn * (qq + 1)])
                    nc.scalar.dma_start(out=seT_bf[:, qn * qq:qn * (qq + 1)],
                                        in_=seqT.ap()[:, qn * qq:qn * (qq + 1)])
                # aT = Wa^T h for local nodes (feature-major, first quarter)
                for hh in range(n_loc // 512):
                    psa = p1p.tile([128, 512], F32, tag="psa")
                    nc.tensor.matmul(out=psa[:], lhsT=s_wa[:],
                                     rhs=nhT_bf[:, 512 * hh:512 * (hh + 1)],
                                     start=True, stop=True)
                    nc.scalar.activation(out=s_aT[:, 512 * hh:512 * (hh + 1)],
                                         in_=psa[:], func=COPY)
                # table rows: node-major [128, 256] per block, u | v halves
                tstage = p1s.tile([128, gblk, 256], BF, tag="tstage")
                for b in range(gblk):
                    ps = p1p.tile([128, 256], F32, tag="tps")
                    nc.tensor.matmul(out=ps[:, 0:128],
                                     lhsT=nhT_bf[:, 128 * b:128 * (b + 1)],
                                     rhs=s_wb[:], start=True, stop=True)
                    nc.tensor.matmul(out=ps[:, 128:256],
                                     lhsT=seT_bf[:, 128 * b:128 * (b + 1)],
                                     rhs=s_wd[:], start=True, stop=True)
                    if b % 2 == 0:
                        nc.scalar.activation(out=tstage[:, b, :], in_=ps[:],
                                             func=COPY)
                    else:
                        nc.vector.tensor_copy(out=tstage[:, b, :], in_=ps[:])
                    if b % 16 == 15:
                        g0 = b - 15
                        tslice = table[128 * g0:128 * (g0 + 16), :]
                        tslice = tslice.rearrange("(b p) f -> p b f", p=128)
                        nc.scalar.dma_start(out=tslice,
                                            in_=tstage[:, g0:g0 + 16, :])

            # ---- phase 2: main edge loop, k-major chunks ----
            with tc.tile_pool(name="p2g", bufs=7) as p2g, \
                 tc.tile_pool(name="p2e", bufs=2) as p2e, \
                 tc.tile_pool(name="p2d", bufs=3) as p2d, \
                 tc.tile_pool(name="p2t2", bufs=3) as p2t2, \
                 tc.tile_pool(name="p2t4", bufs=2) as p2t4, \
                 tc.tile_pool(name="p2r", bufs=2) as p2r, \
                 tc.tile_pool(name="pp1", bufs=2, space="PSUM") as pp1, \
                 tc.tile_pool(name="ppw", bufs=2, space="PSUM") as ppw:
                for ch in range(n_ch):
                    g = p2g.tile([128, N_SUB, 256], BF, tag="g")
                    nc.gpsimd.dma_gather(
                        out_ap=g[:],
                        in_ap=table[:],
                        idxs_ap=s_idx[:, (CH_E // 16) * ch:
                                      (CH_E // 16) * (ch + 1)],
                        num_idxs=CH_E,
                        num_idxs_reg=CH_E,
                        elem_size=256,
                        single_packet=False,
                        queue_num=1 + ch % 3,
                    )
                    e = p2e.tile([128, CH_E], BF, tag="e")
                    nc.sync.dma_start(out=e[:],
                                      in_=edge_hT.ap()[:, CH_E * ch:
                                                       CH_E * (ch + 1)])
                    # diag(mask) tiles for this chunk's 24 subtiles
                    dg = p2d.tile([128, N_SUB, 128], BF, tag="dg")
                    for s in range(N_SUB):
                        nc.vector.tensor_scalar_mul(
                            out=dg[:, s, :], in0=s_id[:],
                            scalar1=s_maskc[:, N_SUB * ch + s:
                                            N_SUB * ch + s + 1])

                    t4 = p2t4.tile([128, CH_E], BF, tag="t4")
                    na = s_aT[:, CH_NODES * ch:CH_NODES * (ch + 1)]
                    rep = bass.AP(tensor=na.tensor, offset=na.offset,
                                  ap=[na.ap[0], [0, 8], na.ap[1]])
                    for hc in range(2):
                        ps1 = pp1.tile([128, 3, 512], F32, tag="ps1")
                        e0 = HC_E * hc  # edge col offset within chunk
                        for b in range(3):
                            nc.tensor.matmul(
                                out=ps1[:, b, :], lhsT=s_wc[:],
                                rhs=e[:, e0 + 512 * b:e0 + 512 * (b + 1)],
                                start=True, stop=False)
                        for b in range(3):
                            nc.tensor.matmul(out=ps1[:, b, :], lhsT=s_id[:],
                                             rhs=rep, start=False, stop=False)
                        for sub in range(12):
                            gsub = 12 * hc + sub
                            bank = sub // 4
                            col = 128 * (sub % 4)
                            nc.tensor.matmul(
                                out=ps1[:, bank, col:col + 128],
                                lhsT=g[:, gsub, 0:128], rhs=s_id[:],
                                start=False, stop=False)
                            nc.tensor.matmul(
                                out=ps1[:, bank, col:col + 128],
                                lhsT=g[:, gsub, 128:256], rhs=dg[:, gsub, :],
                                start=False, stop=True)
                        t2 = p2t2.tile([128, HC_E], BF, tag="t2")
                        nc.scalar.activation(out=t2[:], in_=ps1[:], func=GELU,
                                             bias=s_mb1c[:])
                        for b in range(3):
                            psw = ppw.tile([128, 512], F32, tag="psw")
                            nc.tensor.matmul(out=psw[:], lhsT=s_w2[:],
                                             rhs=t2[:, 512 * b:512 * (b + 1)],
                                             start=True, stop=True)
                            nc.scalar.activation(
                                out=t4[:, e0 + 512 * b:e0 + 512 * (b + 1)],
                                in_=psw[:], func=GELU, bias=s_mb2c[:])
                    # k-major tree reduce: 48 -> 24 -> 12 -> 6 -> 3 -> 1
                    r1 = p2r.tile([128, HC_E], BF, tag="r1")
                    nc.vector.tensor_add(out=r1[:], in0=t4[:, 0:HC_E],
                                         in1=t4[:, HC_E:CH_E])
                    nc.vector.tensor_add(out=r1[:, 0:768], in0=r1[:, 0:768],
                                         in1=r1[:, 768:1536])
                    nc.vector.tensor_add(out=r1[:, 0:384], in0=r1[:, 0:384],
                                         in1=r1[:, 384:768])
                    nc.vector.tensor_add(out=r1[:, 0:192], in0=r1[:, 0:192],
                                         in1=r1[:, 192:384])
                    nc.vector.tensor_add(out=r1[:, 0:64], in0=r1[:, 0:64],
                                         in1=r1[:, 64:128])
                    nc.vector.tensor_add(out=s_aggTb[:, CH_NODES * ch:
                                                     CH_NODES * (ch + 1)],
                                         in0=r1[:, 0:64], in1=r1[:, 128:192])

            # ---- phase 3: mm3, LN1, FF, LN2, output (stage-batched) ----
            with tc.tile_pool(name="p3s", bufs=8) as p3s, \
                 tc.tile_pool(name="p3u", bufs=4) as p3u, \
                 tc.tile_pool(name="p3o", bufs=2) as p3o, \
                 tc.tile_pool(name="pp3", bufs=1, space="PSUM") as pp3, \
                 tc.tile_pool(name="ppm", bufs=1, space="PSUM") as ppm, \
                 tc.tile_pool(name="ppf", bufs=2, space="PSUM") as ppf, \
                 tc.tile_pool(name="ppf2", bufs=2, space="PSUM") as ppf2:
                nh_half = n_loc // 512
                # mm3 + 48*mb3 -> a2Tb bf16
                for hh in range(nh_half):
                    psm = ppm.tile([128, 512], F32, tag="psm")
                    nc.tensor.matmul(out=psm[:], lhsT=s_w3[:],
                                     rhs=s_aggTb[:, 512 * hh:512 * (hh + 1)],
                                     start=True, stop=True)
                    nc.scalar.activation(out=s_a2Tb[:, 512 * hh:512 * (hh + 1)],
                                         in_=psm[:], func=IDENT,
                                         bias=s_mb3x48[:])
                # LN1, stage-batched across the 8 blocks
                pstT = pp3.tile([128, nblk, 128], F32, tag="pstT")
                for b in range(nblk):
                    nc.tensor.matmul(out=pstT[:, b, :],
                                     lhsT=s_a2Tb[:, 128 * b:128 * (b + 1)],
                                     rhs=s_id[:], start=True, stop=True)
                x1s, mvs, rstds = [], [], []
                for b in range(nblk):
                    x1 = p3s.tile([128, 128], F32, tag="x1")
                    nc.vector.tensor_add(out=x1[:], in0=pstT[:, b, :],
                                         in1=s_nhl[:, b, :])
                    x1s.append(x1)
                for b in range(nblk):
                    st = p3s.tile([128, 6], F32, tag="st")
                    nc.vector.bn_stats(out=st[:], in_=x1s[b][:])
                    mv = p3s.tile([128, 2], F32, tag="mv")
                    nc.vector.bn_aggr(out=mv[:], in_=st[:])
                    mvs.append(mv)
                for b in range(nblk):
                    sd = p3s.tile([128, 1], F32, tag="sd")
                    nc.scalar.activation(out=sd[:], in_=mvs[b][:, 1:2],
                                         func=SQRT, bias=s_eps[:])
                    rstd = p3s.tile([128, 1], F32, tag="rstd")
                    nc.vector.reciprocal(out=rstd[:], in_=sd[:])
                    rstds.append(rstd)
                for b in range(nblk):
                    xn = p3s.tile([128, 128], BF, tag="xn")
                    nc.vector.tensor_scalar(out=xn[:], in0=x1s[b][:],
                                            scalar1=mvs[b][:, 0:1],
                                            scalar2=rstds[b][:],
                                            op0=SUB, op1=MUL)
                    tb1 = p3s.tile([128, 128], BF, tag="tb1")
                    nc.vector.tensor_mul(out=tb1[:], in0=xn[:], in1=s_g1bc[:])
                    nc.vector.tensor_add(out=s_h1rm[:, b, :], in0=tb1[:],
                                         in1=s_b1bc[:])
                psTT = pp3.tile([128, nblk, 128], F32, tag="pstT")
                for b in range(nblk):
                    nc.tensor.matmul(out=psTT[:, b, :], lhsT=s_h1rm[:, b, :],
                                     rhs=s_id[:], start=True, stop=True)
                for b in range(0, nblk, 4):
                    nc.scalar.activation(
                        out=s_h1T[:, 128 * b:128 * (b + 4)],
                        in_=psTT[:, b:b + 4, :], func=COPY)
                # FF + LN2 per 512-node half
                for hh in range(nh_half):
                    us = []
                    for fc in range(4):
                        psf = ppf.tile([128, 512], F32, tag="psf")
                        nc.tensor.matmul(out=psf[:],
                                         lhsT=s_fw1[:, 128 * fc:128 * (fc + 1)],
                                         rhs=s_h1T[:, 512 * hh:512 * (hh + 1)],
                                         start=True, stop=True)
                        u = p3u.tile([128, 512], BF, tag=f"u{fc}")
                        nc.scalar.activation(out=u[:], in_=psf[:], func=GELU,
                                             bias=s_fb1c[:, fc:fc + 1])
                        us.append(u)
                    psf2 = ppf2.tile([128, 512], F32, tag="psf2")
                    for fc in range(4):
                        nc.tensor.matmul(out=psf2[:], lhsT=s_fw2[:, fc, :],
                                         rhs=us[fc][:], start=(fc == 0),
                                         stop=(fc == 3))
                    ffTs, psr2s, x2s, mv2s, rstd2s = [], [], [], [], []
                    for j in range(4):
                        ffT = p3s.tile([128, 128], BF, tag=f"ffT{j}")
                        nc.scalar.activation(out=ffT[:],
                                             in_=psf2[:, 128 * j:128 * (j + 1)],
                                             func=IDENT, bias=s_fb2c[:])
                        ffTs.append(ffT)
                    psr2T = pp3.tile([128, 4, 128], F32, tag="psr2T")
                    for j in range(4):
                        nc.tensor.matmul(out=psr2T[:, j, :], lhsT=ffTs[j][:],
                                         rhs=s_id[:], start=True, stop=True)
                    for j in range(4):
                        b = 4 * hh + j
                        x2 = p3s.tile([128, 128], F32, tag=f"x2{j}")
                        nc.vector.tensor_add(out=x2[:], in0=psr2T[:, j, :],
                                             in1=s_h1rm[:, b, :])
                        x2s.append(x2)
                    for j in range(4):
                        st2 = p3s.tile([128, 6], F32, tag="st2")
                        nc.vector.bn_stats(out=st2[:], in_=x2s[j][:])
                        mv2 = p3s.tile([128, 2], F32, tag=f"mv2{j}")
                        nc.vector.bn_aggr(out=mv2[:], in_=st2[:])
                        mv2s.append(mv2)
                    for j in range(4):
                        sd2 = p3s.tile([128, 1], F32, tag="sd2")
                        nc.scalar.activation(out=sd2[:], in_=mv2s[j][:, 1:2],
                                             func=SQRT, bias=s_eps[:])
                        rstd2 = p3s.tile([128, 1], F32, tag=f"rstd2{j}")
                        nc.vector.reciprocal(out=rstd2[:], in_=sd2[:])
                        rstd2s.append(rstd2)
                    ob = p3o.tile([128, 4, 128], F32, tag="ob")
                    for j in range(4):
                        xn2 = p3s.tile([128, 128], F32, tag="xn2")
                        nc.vector.tensor_scalar(out=xn2[:], in0=x2s[j][:],
                                                scalar1=mv2s[j][:, 0:1],
                                                scalar2=rstd2s[j][:],
                                                op0=SUB, op1=MUL)
                        tg = p3s.tile([128, 128], F32, tag="tg")
                        nc.vector.tensor_mul(out=tg[:], in0=xn2[:],
                                             in1=s_g2bc[:])
                        nc.vector.tensor_add(out=ob[:, j, :], in0=tg[:],
                                             in1=s_b2bc[:])
                    oslice = out.ap()[512 * hh:512 * (hh + 1), :]
                    oslice = oslice.rearrange("(j p) f -> p j f", p=128)
                    nc.sync.dma_start(out=oslice, in_=ob[:])

    nc.compile()
    return nc


def prep_core_inputs(inputs, n_glob, n_loc, core):
    """Host-side layout prep for one core: slicing, k-major reorder within
    chunks, transposes, dtype casts, tiny constant broadcasts. No kernel math
    (no indexing of data tensors by edge_idx) is done on the host."""
    f32 = np.float32
    n0 = core * n_loc
    E = n_loc * KK
    n_ch = E // CH_E

    def kmajor(x):
        # x: [n_loc, KK, ...] -> per 64-node chunk: [KK, 64, ...] -> flat E
        tail = x.shape[2:]
        x = x.reshape(n_ch, CH_NODES, KK, *tail)
        x = x.transpose(0, 2, 1, *range(3, 3 + len(tail)))
        return np.ascontiguousarray(x.reshape(E, *tail))

    eh = kmajor(inputs["edge_h"][n0:n0 + n_loc])          # [E, H] k-major
    eh = np.ascontiguousarray(eh.T).astype(BF16)          # [H, E]
    j = (inputs["edge_idx"][n0:n0 + n_loc].astype(np.int64) - n0) % n_glob
    j = kmajor(j)                                         # [E] k-major
    idx16 = np.tile(np.ascontiguousarray(j.reshape(E // 16, 16).T), (8, 1)
                    ).astype(np.int16)
    m = kmajor(inputs["ar_mask"][n0:n0 + n_loc])          # [E] k-major
    maskc = np.ascontiguousarray(m.reshape(E // 128, 128).T).astype(f32)
    node_hT = np.ascontiguousarray(
        np.roll(inputs["node_h"], -n0, axis=0).T).astype(BF16)
    seqT = np.ascontiguousarray(
        np.roll(inputs["seq_emb"], -n0, axis=0).T).astype(BF16)
    nhl = np.ascontiguousarray(
        inputs["node_h"][n0:n0 + n_loc].reshape(n_loc // 128, 128, H)
        .transpose(1, 0, 2)).astype(f32)
    mW1 = inputs["mW1"]
    d = {
        "edge_hT": eh, "idx16": idx16, "maskc": maskc,
        "node_hT": node_hT, "seqT": seqT, "nhl": nhl,
        "wa": mW1[0:128].astype(BF16), "wb": mW1[128:256].astype(BF16),
        "wc": mW1[256:384].astype(BF16),
        "wd": mW1[384:512].astype(BF16),
        "w2": inputs["mW2"].astype(BF16), "w3": inputs["mW3"].astype(BF16),
        "fw1": inputs["fW1"].astype(BF16),
        "fw2": np.ascontiguousarray(
            inputs["fW2"].reshape(4, 128, H).transpose(1, 0, 2)).astype(BF16),
        "ident": np.eye(128, dtype=BF16),
        "mb1c": inputs["mb1"].reshape(H, 1).astype(f32),
        "mb2c": inputs["mb2"].reshape(H, 1).astype(f32),
        "mb3x48": (inputs["mb3"] * KK).reshape(H, 1).astype(f32),
        "fb1c": np.ascontiguousarray(
            inputs["fb1"].reshape(4, 128).T).astype(f32),
        "fb2c": inputs["fb2"].reshape(H, 1).astype(f32),
        "g1bc": np.tile(inputs["g1"][None, :], (128, 1)).astype(BF16),
        "b1bc": np.tile(inputs["b1"][None, :], (128, 1)).astype(BF16),
        "g2bc": np.tile(inputs["g2"][None, :], (128, 1)).astype(f32),
        "b2bc": np.tile(inputs["b2"][None, :], (128, 1)).astype(f32),
    }
    return d


_NC_CACHE = {}


def kernel(**inputs):
    inputs = {k: np.asarray(v) for k, v in inputs.items()}
    n_glob = inputs["node_h"].shape[0]
    n_loc = n_glob // N_CORES
    key = (n_glob, n_loc)
    if key not in _NC_CACHE:
        _NC_CACHE[key] = build_nc(n_glob, n_loc, N_CORES)
    nc = _NC_CACHE[key]
    in_maps = [prep_core_inputs(inputs, n_glob, n_loc, c)
               for c in range(N_CORES)]
    res = bass_utils.run_bass_kernel_spmd(nc, in_maps,
                                          core_ids=list(range(N_CORES)))
    return np.concatenate([res.results[c]["out"] for c in range(N_CORES)],
                          axis=0).astype(np.float32)


# revision 26
# speedup vs baseline: 2.7676x; 1.2833x over previous
"""Trainium2 Bass kernel for nn_DecoderLayer (GNN message passing decoder layer).

Math (per reference):
  seq_j = seq_emb[edge_idx] * ar_mask[..., None]
  x = concat([h_i, h_j, edge_h, seq_j], -1)            # [res,k,4h]
  msg = gelu(x @ mW1 + mb1); msg = gelu(msg @ mW2 + mb2); msg = msg @ mW3 + mb3
  agg = msg.sum(1)
  h = LN(node_h + agg) * g1 + b1
  ff = gelu(h @ fW1 + fb1) @ fW2 + fb2
  h = LN(h + ff) * g2 + b2

Strategy (8-way data parallel over the residue dim, no collectives):
  - mm1 decomposed: x@mW1 = h_i@Wa + h_j@Wb + edge_h@Wc + seq_j@Wd.
    h_j@Wb and seq_emb@Wd are precomputed per global node into a fused bf16
    gather table [8192, 256] in DRAM; per-edge rows fetched with dma_gather.
  - dma_gather descriptor generation runs on the GpSimd Q7 core-pair selected
    by queue_num; queues 1-3 run concurrently on three disjoint pairs
    (num_swdge_queues=4), tripling gather throughput vs one queue.
  - Edges are k-major within 3072-edge chunks (64 nodes x 48 k): col=k*64+n.
    Aligns 512-col matmul slices with the per-node h_i@Wa broadcast AP and
    makes the k-reduction a log-tree of dense bf16 adds on DVE.
  - ar_mask folded into the PE transpose of the gathered seq half:
    rhs = diag(mask), host-built and DMA'd per chunk. No mask multiply pass.
  - k-reduction before mm3 (linearity): 48x less mm3 work; done as one
    strided DVE reduce per chunk.
  - mm2 (+gelu2+reduce) of chunk N is issued interleaved into chunk N+1's
    mm1 stream so ACT always has ready work and PE never waits on gelu1.
  - constants packed into two blob tensors (2 DMAs instead of ~24), table
    built 4 blocks per PSUM tile and written to DRAM in 8 batched DMAs.
"""

import os
import sys

sys.path.insert(0, "/opt/trn_rl_repo")

import numpy as np
import ml_dtypes

import concourse.bacc as bacc
import concourse.bass as bass
import concourse.mybir as mybir
import concourse.tile as tile
from concourse import bass_utils

BF16 = ml_dtypes.bfloat16
F32 = mybir.dt.float32
BF = mybir.dt.bfloat16
I16 = mybir.dt.int16
F8 = mybir.dt.float8e4

RES, KK, H = 8192, 48, 128
N_CORES = 8
CH_NODES = 64                 # nodes per chunk
CH_E = CH_NODES * KK          # 3072 edges per chunk
HC_E = CH_E // 2              # 1536 edges per half-chunk
N_SUB = CH_E // 128           # 24 subtiles of 128 edges per chunk


def build_nc(n_glob, n_loc, num_devices):
    E = n_loc * KK
    n_ch = E // CH_E           # 16 chunks
    nblk = n_loc // 128        # 8 local node blocks
    gblk = n_glob // 128       # 64 global node blocks

    nc = bacc.Bacc("TRN2", target_bir_lowering=False, debug=False,
                   num_devices=num_devices, num_swdge_queues=4)

    def din(name, shape, dt):
        return nc.dram_tensor(name, shape, dt, kind="ExternalInput")

    edge_hT = din("edge_hT", [H, E], BF)            # k-major per chunk
    idx16 = din("idx16", [128, E // 16], I16)       # k-major per chunk
    node_hT = din("node_hT", [H, n_glob], BF)       # rotated: local first
    seqT = din("seqT", [H, n_glob], BF)
    blob_bf = din("blob_bf", [128, 2176], BF)       # packed bf16 constants
    blob_f32 = din("blob_f32", [128, 1288], F32)    # packed f32 constants
    diagc = din("diagc", [128, E // 128, 128], BF)  # host-built diag(mask)
    out = nc.dram_tensor("out", [n_loc, H], F32, kind="ExternalOutput")

    GELU = mybir.ActivationFunctionType.Gelu
    IDENT = mybir.ActivationFunctionType.Identity
    COPY = mybir.ActivationFunctionType.Copy
    SQRT = mybir.ActivationFunctionType.Sqrt
    SUB = mybir.AluOpType.subtract
    MUL = mybir.AluOpType.mult

    with tile.TileContext(nc) as tc:
        with tc.tile_pool(name="singles", bufs=1) as sg, \
             tc.tile_pool(name="dram", bufs=1, space="DRAM") as dp:
            # ---- resident constants: two packed blobs + idx ----
            # blob_bf first (weights gate all phase-1 compute); idx is only
            # needed at the first gather, so it loads last.
            s_bb = sg.tile([128, 2176], BF)
            nc.sync.dma_start(out=s_bb[:], in_=blob_bf.ap())
            s_bf = sg.tile([128, 1288], F32)
            nc.scalar.dma_start(out=s_bf[:], in_=blob_f32.ap())
            s_idx = sg.tile([128, E // 16], I16)
            s_wa = s_bb[:, 0:128]
            s_wb = s_bb[:, 128:256]
            s_wd = s_bb[:, 256:384]
            s_wc = s_bb[:, 384:512]
            s_wbd = s_bb[:, 128:384]
            s_w2 = s_bb[:, 512:640]
            s_w3 = s_bb[:, 640:768]
            s_fw1 = s_bb[:, 768:1280]
            s_fw2 = s_bb[:, 1280:1792].rearrange("p (a b) -> p a b", a=4)
            s_id = s_bb[:, 1792:1920]
            s_g1bc = s_bb[:, 1920:2048]
            s_b1bc = s_bb[:, 2048:2176]
            s_mb1c = s_bf[:, 0:1]
            s_mb2c = s_bf[:, 1:2]
            s_mb3x48 = s_bf[:, 2:3]
            s_fb1c = s_bf[:, 3:7]
            s_fb2c = s_bf[:, 7:8]
            s_g2bc = s_bf[:, 8:136]
            s_b2bc = s_bf[:, 136:264]
            s_nhl = s_bf[:, 264:1288].rearrange("p (a b) -> p a b", a=nblk)
            s_eps = sg.tile([128, 1], F32)
            nc.vector.memset(s_eps[:], 1e-5)

            s_aT = sg.tile([128, n_loc], BF)        # (Wa^T h_i) per local node
            s_aggTb = sg.tile([128, n_loc], BF)     # k-sum of msg2, fm bf16
            s_a2Tb = sg.tile([128, n_loc], BF)
            s_h1T = sg.tile([128, n_loc], BF)
            s_h1rm = sg.tile([128, nblk, H], BF)

            table = dp.tile([n_glob, 256], BF)

            # ---- phase 1: gather table + Wa precompute ----
            # nhT/seqT loaded in quarters so block-0 compute starts early;
            # table staged in SBUF and written to DRAM in 4 batched DMAs.
            with tc.tile_pool(name="p1s", bufs=1) as p1s, \
                 tc.tile_pool(name="p1c", bufs=4) as p1c, \
                 tc.tile_pool(name="p1p", bufs=2, space="PSUM") as p1p, \
                 tc.tile_pool(name="p1p2", bufs=3, space="PSUM") as p1p2:
                nhT_bf = p1s.tile([128, n_glob], BF, tag="big1")
                seT_bf = p1s.tile([128, n_glob], BF, tag="big2")
                qn = n_glob // 4
                for qq in range(4):
                    nc.sync.dma_start(out=nhT_bf[:, qn * qq:qn * (qq + 1)],
                                      in_=node_hT.ap()[:, qn * qq:qn * (qq + 1)])
                    nc.scalar.dma_start(out=seT_bf[:, qn * qq:qn * (qq + 1)],
                                        in_=seqT.ap()[:, qn * qq:qn * (qq + 1)])
                nc.sync.dma_start(out=s_idx[:], in_=idx16.ap())
                # aT = Wa^T h for local nodes (feature-major, first quarter)
                for hh in range(n_loc // 512):
                    psa = p1p.tile([128, 512], F32, tag="psa")
                    nc.tensor.matmul(out=psa[:], lhsT=s_wa[:],
                                     rhs=nhT_bf[:, 512 * hh:512 * (hh + 1)],
                                     start=True, stop=True)
                    nc.scalar.activation(out=s_aT[:, 512 * hh:512 * (hh + 1)],
                                         in_=psa[:], func=COPY)
                # table rows: node-major, built 4 blocks (512 nodes) per
                # PSUM tile so the copy/write pipeline amortizes hop latency.
                tstage = p1s.tile([128, gblk, 256], BF, tag="tstage")
                for gq in range(gblk // 4):
                    ps4 = p1p2.tile([128, 4, 256], F32, tag="tps4")
                    for j in range(4):
                        b = 4 * gq + j
                        nc.tensor.matmul(out=ps4[:, j, 0:128],
                                         lhsT=nhT_bf[:, 128 * b:128 * (b + 1)],
                                         rhs=s_wb[:], start=True, stop=True)
                        nc.tensor.matmul(out=ps4[:, j, 128:256],
                                         lhsT=seT_bf[:, 128 * b:128 * (b + 1)],
                                         rhs=s_wd[:], start=True, stop=True)
                    if gq % 2 == 0:
                        nc.scalar.activation(out=tstage[:, 4 * gq:4 * gq + 4, :],
                                             in_=ps4[:], func=COPY)
                    else:
                        nc.vector.tensor_copy(out=tstage[:, 4 * gq:4 * gq + 4, :],
                                              in_=ps4[:])
                    if gq % 2 == 1:
                        g0 = 4 * (gq - 1)
                        tslice = table[128 * g0:128 * (g0 + 8), :]
                        tslice = tslice.rearrange("(b p) f -> p b f", p=128)
                        nc.sync.dma_start(out=tslice,
                                          in_=tstage[:, g0:g0 + 8, :])

            # ---- phase 2: main edge loop, k-major chunks ----
            with tc.tile_pool(name="p2g", bufs=8) as p2g, \
                 tc.tile_pool(name="p2e", bufs=3) as p2e, \
                 tc.tile_pool(name="p2d", bufs=3) as p2d, \
                 tc.tile_pool(name="p2t2", bufs=4) as p2t2, \
                 tc.tile_pool(name="p2r", bufs=2) as p2r, \
                 tc.tile_pool(name="p2t4", bufs=3) as p2t4, \
                 tc.tile_pool(name="pp1", bufs=2, space="PSUM") as pp1, \
                 tc.tile_pool(name="ppw", bufs=2, space="PSUM") as ppw:
                def w2_stage(pend, hc):
                    t2s, t4p = pend
                    e0 = HC_E * hc
                    for b in range(3):
                        psw = ppw.tile([128, 512], F32, tag="psw",
                                       name=f"psw{hc}{b}")
                        nc.tensor.matmul(out=psw[:], lhsT=s_w2[:],
                                         rhs=t2s[hc][:, 512 * b:512 * (b + 1)],
                                         start=True, stop=True)
                        nc.scalar.activation(
                            out=t4p[:, e0 + 512 * b:e0 + 512 * (b + 1)],
                            in_=psw[:], func=GELU, bias=s_mb2c[:])

                def reduce_stage(pendx, _unused, chp):
                    # dense log-tree over the k-major layout: level 1 frees
                    # the t4 buffer immediately; dense bf16 adds hit the DVE
                    # 2x/4x modes (the strided one-op reduce suffers SBUF
                    # bank conflicts at stride 128B and holds t4 5+ us).
                    _, t4p = pendx[0]
                    r1 = p2r.tile([128, HC_E], BF, tag="r1")
                    nc.vector.tensor_add(out=r1[:], in0=t4p[:, 0:HC_E],
                                         in1=t4p[:, HC_E:CH_E])
                    nc.vector.tensor_add(out=r1[:, 0:768], in0=r1[:, 0:768],
                                         in1=r1[:, 768:1536])
                    nc.vector.tensor_add(out=r1[:, 0:384], in0=r1[:, 0:384],
                                         in1=r1[:, 384:768])
                    nc.vector.tensor_add(out=r1[:, 0:192], in0=r1[:, 0:192],
                                         in1=r1[:, 192:384])
                    nc.vector.tensor_add(out=r1[:, 0:64], in0=r1[:, 0:64],
                                         in1=r1[:, 64:128])
                    nc.vector.tensor_add(out=s_aggTb[:, CH_NODES * chp:
                                                     CH_NODES * (chp + 1)],
                                         in0=r1[:, 0:64], in1=r1[:, 128:192])

                pend = None
                for ch in range(n_ch):
                    g = p2g.tile([128, N_SUB, 256], BF, tag="g")
                    nc.gpsimd.dma_gather(
                        out_ap=g[:],
                        in_ap=table[:],
                        idxs_ap=s_idx[:, (CH_E // 16) * ch:
                                      (CH_E // 16) * (ch + 1)],
                        num_idxs=CH_E,
                        num_idxs_reg=CH_E,
                        elem_size=256,
                        single_packet=False,
                        queue_num=(1 + ch % 3) if ch < n_ch - 1 else 0,
                    )
                    e = p2e.tile([128, CH_E], BF, tag="e")
                    nc.sync.dma_start(out=e[:],
                                      in_=edge_hT.ap()[:, CH_E * ch:
                                                       CH_E * (ch + 1)])
                    dg = p2d.tile([128, N_SUB, 128], BF, tag="dg")
                    nc.scalar.dma_start(out=dg[:],
                                        in_=diagc.ap()[:, N_SUB * ch:
                                                       N_SUB * (ch + 1), :])

                    t4 = p2t4.tile([128, CH_E], BF, tag="t4")
                    na = s_aT[:, CH_NODES * ch:CH_NODES * (ch + 1)]
                    rep = bass.AP(tensor=na.tensor, offset=na.offset,
                                  ap=[na.ap[0], [0, 8], na.ap[1]])
                    t2s = []
                    for hc in range(2):
                        ps1 = pp1.tile([128, 3, 512], F32, tag="ps1")
                        e0 = HC_E * hc  # edge col offset within chunk
                        for b in range(3):
                            nc.tensor.matmul(
                                out=ps1[:, b, :], lhsT=s_wc[:],
                                rhs=e[:, e0 + 512 * b:e0 + 512 * (b + 1)],
                                start=True, stop=False)
                        for b in range(3):
                            nc.tensor.matmul(out=ps1[:, b, :], lhsT=s_id[:],
                                             rhs=rep, start=False, stop=False)
                        for sub in range(12):
                            gsub = 12 * hc + sub
                            bank = sub // 4
                            col = 128 * (sub % 4)
                            nc.tensor.matmul(
                                out=ps1[:, bank, col:col + 128],
                                lhsT=g[:, gsub, 0:128], rhs=s_id[:],
                                start=False, stop=False)
                            nc.tensor.matmul(
                                out=ps1[:, bank, col:col + 128],
                                lhsT=g[:, gsub, 128:256], rhs=dg[:, gsub, :],
                                start=False, stop=True)
                        # previous chunk's w2 stage first: its gelu2 inputs
                        # are ready, so ACT drains them while PE works here.
                        if pend is not None:
                            w2_stage(pend[0], hc)
                            if hc == 1:
                                reduce_stage((pend[0],), None, pend[1])
                        t2 = p2t2.tile([128, HC_E], BF, tag="t2")
                        nc.scalar.activation(out=t2[:], in_=ps1[:], func=GELU,
                                             bias=s_mb1c[:])
                        t2s.append(t2)
                    pend = ((t2s, t4), ch)
                w2_stage(pend[0], 0)
                w2_stage(pend[0], 1)
                reduce_stage((pend[0],), None, pend[1])

            # ---- phase 3: mm3, LN1, FF, LN2, output (stage-batched) ----
            with tc.tile_pool(name="p3s", bufs=8) as p3s, \
                 tc.tile_pool(name="p3u", bufs=4) as p3u, \
                 tc.tile_pool(name="p3o", bufs=2) as p3o, \
                 tc.tile_pool(name="pp3", bufs=1, space="PSUM") as pp3, \
                 tc.tile_pool(name="ppm", bufs=1, space="PSUM") as ppm, \
                 tc.tile_pool(name="ppf", bufs=2, space="PSUM") as ppf, \
                 tc.tile_pool(name="ppf2", bufs=2, space="PSUM") as ppf2:
                nh_half = n_loc // 512
                # mm3 + 48*mb3 -> a2Tb bf16
                for hh in range(nh_half):
                    psm = ppm.tile([128, 512], F32, tag="psm")
                    nc.tensor.matmul(out=psm[:], lhsT=s_w3[:],
                                     rhs=s_aggTb[:, 512 * hh:512 * (hh + 1)],
                                     start=True, stop=True)
                    nc.scalar.activation(out=s_a2Tb[:, 512 * hh:512 * (hh + 1)],
                                         in_=psm[:], func=IDENT,
                                         bias=s_mb3x48[:])
                # LN1, stage-batched across the 8 blocks
                pstT = pp3.tile([128, nblk, 128], F32, tag="pstT")
                for b in range(nblk):
                    nc.tensor.matmul(out=pstT[:, b, :],
                                     lhsT=s_a2Tb[:, 128 * b:128 * (b + 1)],
                                     rhs=s_id[:], start=True, stop=True)
                x1a = p3s.tile([128, nblk, 128], F32, tag="x1a")
                nc.vector.tensor_add(out=x1a[:], in0=pstT[:], in1=s_nhl[:])
                sta = p3s.tile([128, nblk, 6], F32, tag="sta")
                mva = p3s.tile([128, nblk, 2], F32, tag="mva")
                for b in range(nblk):
                    nc.vector.bn_stats(out=sta[:, b, :], in_=x1a[:, b, :])
                    nc.vector.bn_aggr(out=mva[:, b, :], in_=sta[:, b, :])
                sda = p3s.tile([128, nblk], F32, tag="sda")
                nc.scalar.activation(out=sda[:], in_=mva[:, :, 1:2],
                                     func=SQRT, bias=s_eps[:])
                rstda = p3s.tile([128, nblk], F32, tag="rstda")
                nc.vector.reciprocal(out=rstda[:], in_=sda[:])
                xna = p3s.tile([128, nblk, 128], BF, tag="xna")
                for b in range(nblk):
                    nc.vector.tensor_scalar(out=xna[:, b, :],
                                            in0=x1a[:, b, :],
                                            scalar1=mva[:, b, 0:1],
                                            scalar2=rstda[:, b:b + 1],
                                            op0=SUB, op1=MUL)
                g1rep = bass.AP(tensor=s_g1bc.tensor, offset=s_g1bc.offset,
                                ap=[s_g1bc.ap[0], [0, nblk], s_g1bc.ap[1]])
                b1rep = bass.AP(tensor=s_b1bc.tensor, offset=s_b1bc.offset,
                                ap=[s_b1bc.ap[0], [0, nblk], s_b1bc.ap[1]])
                tb1a = p3s.tile([128, nblk, 128], BF, tag="tb1a")
                nc.vector.tensor_mul(out=tb1a[:], in0=xna[:], in1=g1rep)
                nc.vector.tensor_add(out=s_h1rm[:], in0=tb1a[:], in1=b1rep)
                psTT = pp3.tile([128, nblk, 128], F32, tag="pstT")
                for b in range(nblk):
                    nc.tensor.matmul(out=psTT[:, b, :], lhsT=s_h1rm[:, b, :],
                                     rhs=s_id[:], start=True, stop=True)
                for b in range(0, nblk, 4):
                    nc.scalar.activation(
                        out=s_h1T[:, 128 * b:128 * (b + 4)],
                        in_=psTT[:, b:b + 4, :], func=COPY)
                # FF + LN2 per 512-node half
                for hh in range(nh_half):
                    us = []
                    for fc in range(4):
                        psf = ppf.tile([128, 512], F32, tag="psf")
                        nc.tensor.matmul(out=psf[:],
                                         lhsT=s_fw1[:, 128 * fc:128 * (fc + 1)],
                                         rhs=s_h1T[:, 512 * hh:512 * (hh + 1)],
                                         start=True, stop=True)
                        u = p3u.tile([128, 512], BF, tag=f"u{fc}")
                        nc.scalar.activation(out=u[:], in_=psf[:], func=GELU,
                                             bias=s_fb1c[:, fc:fc + 1])
                        us.append(u)
                    psf2 = ppf2.tile([128, 512], F32, tag="psf2")
                    for fc in range(4):
                        nc.tensor.matmul(out=psf2[:], lhsT=s_fw2[:, fc, :],
                                         rhs=us[fc][:], start=(fc == 0),
                                         stop=(fc == 3))
                    ffT4 = p3s.tile([128, 4, 128], BF, tag="ffT4")
                    nc.scalar.activation(out=ffT4[:], in_=psf2[:],
                                         func=IDENT, bias=s_fb2c[:])
                    psr2T = pp3.tile([128, 4, 128], F32, tag="psr2T")
                    for j in range(4):
                        nc.tensor.matmul(out=psr2T[:, j, :],
                                         lhsT=ffT4[:, j, :],
                                         rhs=s_id[:], start=True, stop=True)
                    x2a = p3s.tile([128, 4, 128], F32, tag="x2a")
                    nc.vector.tensor_add(out=x2a[:], in0=psr2T[:],
                                         in1=s_h1rm[:, 4 * hh:4 * hh + 4, :])
                    st2a = p3s.tile([128, 4, 6], F32, tag="st2a")
                    mv2a = p3s.tile([128, 4, 2], F32, tag="mv2a")
                    for j in range(4):
                        nc.vector.bn_stats(out=st2a[:, j, :], in_=x2a[:, j, :])
                        nc.vector.bn_aggr(out=mv2a[:, j, :], in_=st2a[:, j, :])
                    sd2a = p3s.tile([128, 4], F32, tag="sd2a")
                    nc.scalar.activation(out=sd2a[:], in_=mv2a[:, :, 1:2],
                                         func=SQRT, bias=s_eps[:])
                    rstd2a = p3s.tile([128, 4], F32, tag="rstd2a")
                    nc.vector.reciprocal(out=rstd2a[:], in_=sd2a[:])
                    xn2a = p3s.tile([128, 4, 128], F32, tag="xn2a")
                    for j in range(4):
                        nc.vector.tensor_scalar(out=xn2a[:, j, :],
                                                in0=x2a[:, j, :],
                                                scalar1=mv2a[:, j, 0:1],
                                                scalar2=rstd2a[:, j:j + 1],
                                                op0=SUB, op1=MUL)
                    g2rep = bass.AP(tensor=s_g2bc.tensor, offset=s_g2bc.offset,
                                    ap=[s_g2bc.ap[0], [0, 4], s_g2bc.ap[1]])
                    b2rep = bass.AP(tensor=s_b2bc.tensor, offset=s_b2bc.offset,
                                    ap=[s_b2bc.ap[0], [0, 4], s_b2bc.ap[1]])
                    ob = p3o.tile([128, 4, 128], F32, tag="ob")
                    tga = p3s.tile([128, 4, 128], F32, tag="tga")
                    nc.vector.tensor_mul(out=tga[:], in0=xn2a[:], in1=g2rep)
                    nc.vector.tensor_add(out=ob[:], in0=tga[:], in1=b2rep)
                    oslice = out.ap()[512 * hh:512 * (hh + 1), :]
                    oslice = oslice.rearrange("(j p) f -> p j f", p=128)
                    nc.sync.dma_start(out=oslice, in_=ob[:])

    nc.compile()
    return nc


def prep_core_inputs(inputs, n_glob, n_loc, core):
    """Host-side layout prep for one core: slicing, k-major reorder within
    chunks, transposes, dtype casts, tiny constant broadcasts. No kernel math
    (no indexing of data tensors by edge_idx) is done on the host."""
    f32 = np.float32
    n0 = core * n_loc
    E = n_loc * KK
    n_ch = E // CH_E

    def kmajor(x):
        # x: [n_loc, KK, ...] -> per 64-node chunk: [KK, 64, ...] -> flat E
        tail = x.shape[2:]
        x = x.reshape(n_ch, CH_NODES, KK, *tail)
        x = x.transpose(0, 2, 1, *range(3, 3 + len(tail)))
        return np.ascontiguousarray(x.reshape(E, *tail))

    eh = kmajor(inputs["edge_h"][n0:n0 + n_loc])          # [E, H] k-major
    eh = np.ascontiguousarray(eh.T).astype(BF16)          # [H, E]
    j = (inputs["edge_idx"][n0:n0 + n_loc].astype(np.int64) - n0) % n_glob
    j = kmajor(j)                                         # [E] k-major
    idx16 = np.tile(np.ascontiguousarray(j.reshape(E // 16, 16).T), (8, 1)
                    ).astype(np.int16)
    m = kmajor(inputs["ar_mask"][n0:n0 + n_loc])          # [E] k-major
    # host-built diag(mask) tiles: [128, E/128, 128] with diag per subtile
    nsub = E // 128
    dia = np.zeros((128, nsub, 128), dtype=BF16)
    ar = np.arange(128)
    dia[ar, :, ar] = m.reshape(nsub, 128).T.astype(BF16)
    node_hT = np.ascontiguousarray(
        np.roll(inputs["node_h"], -n0, axis=0).T).astype(BF16)
    seqT = np.ascontiguousarray(
        np.roll(inputs["seq_emb"], -n0, axis=0).T).astype(BF16)
    nhl = np.ascontiguousarray(
        inputs["node_h"][n0:n0 + n_loc].reshape(n_loc // 128, 128, H)
        .transpose(1, 0, 2)).astype(f32)
    mW1 = inputs["mW1"]
    bb = np.zeros((128, 2176), dtype=BF16)
    bb[:, 0:128] = mW1[0:128].astype(BF16)
    bb[:, 128:256] = mW1[128:256].astype(BF16)
    bb[:, 256:384] = mW1[384:512].astype(BF16)
    bb[:, 384:512] = mW1[256:384].astype(BF16)
    bb[:, 512:640] = inputs["mW2"].astype(BF16)
    bb[:, 640:768] = inputs["mW3"].astype(BF16)
    bb[:, 768:1280] = inputs["fW1"].astype(BF16)
    bb[:, 1280:1792] = np.ascontiguousarray(
        inputs["fW2"].reshape(4, 128, H).transpose(1, 0, 2)).reshape(
        128, 512).astype(BF16)
    bb[:, 1792:1920] = np.eye(128, dtype=BF16)
    bb[:, 1920:2048] = np.tile(inputs["g1"][None, :], (128, 1)).astype(BF16)
    bb[:, 2048:2176] = np.tile(inputs["b1"][None, :], (128, 1)).astype(BF16)
    bf = np.zeros((128, 1288), dtype=f32)
    bf[:, 0] = inputs["mb1"].astype(f32)
    bf[:, 1] = inputs["mb2"].astype(f32)
    bf[:, 2] = (inputs["mb3"] * KK).astype(f32)
    bf[:, 3:7] = np.ascontiguousarray(inputs["fb1"].reshape(4, 128).T)
    bf[:, 7] = inputs["fb2"].astype(f32)
    bf[:, 8:136] = np.tile(inputs["g2"][None, :], (128, 1))
    bf[:, 136:264] = np.tile(inputs["b2"][None, :], (128, 1))
    bf[:, 264:1288] = nhl.reshape(128, 1024)
    d = {
        "edge_hT": eh, "idx16": idx16,
        "node_hT": node_hT, "seqT": seqT,
        "blob_bf": bb, "blob_f32": bf, "diagc": dia,
    }
    return d


_NC_CACHE = {}


def kernel(**inputs):
    inputs = {k: np.asarray(v) for k, v in inputs.items()}
    n_glob = inputs["node_h"].shape[0]
    n_loc = n_glob // N_CORES
    key = (n_glob, n_loc)
    if key not in _NC_CACHE:
        _NC_CACHE[key] = build_nc(n_glob, n_loc, N_CORES)
    nc = _NC_CACHE[key]
    in_maps = [prep_core_inputs(inputs, n_glob, n_loc, c)
               for c in range(N_CORES)]
    res = bass_utils.run_bass_kernel_spmd(nc, in_maps,
                                          core_ids=list(range(N_CORES)))
    return np.concatenate([res.results[c]["out"] for c in range(N_CORES)],
                          axis=0).astype(np.float32)
